# revision 37
# baseline (speedup 1.0000x reference)
"""Trainium2 Bass kernel for nn_Dnn_with_Attention (ragged attention-pooled DNN).

Contract: kernel(**inputs) takes FULL unsharded numpy inputs (keys as in
reference.setup_inputs()) and returns the FULL [256, 10] float32 output.

Strategy (data-parallel over utterances, 8 NeuronCores):
  - Host: greedily balance the 256 segments over 8 cores (32 whole segments
    each), gather each core's frames, transpose x to feature-major
    bf16 [128(feat-padded), M_PAD] and build a per-frame one-hot segment
    membership matrix A (bf16).  A row of ones is appended as feature 78 so
    b1 folds into W1.
  - Device (per core): L1 in bf16 (feature-major, [1024, frames]); L2/L3/L4
    run in fp8 e4m3 with MatmulPerfMode.DoubleRow (two 128-K slices per
    instruction at 0.5 cycles/row, ~4x the f32r rate).  Weights W2/W3/W4 are
    host-quantized to e4m3; inter-layer activations are written as e4m3
    directly by the relu ops.  L4 produces frame-major h4 in bf16; b4 is
    added via a DoubleRow matmul against a host-packed (hi, lo) e4m3 pair so
    the quantization error cancels.  Scores: GpSimd computes h4 * W5 (SBUF
    only; GPSIMD cannot touch PSUM), DVE reduces the innermost axis, Scalar
    takes the exp; e = max(exp(score + b5), 1) folds the relu.  Segment
    softmax pooling is small PE matmuls E.T @ h4 (E = A * e, bf16)
    accumulated into persistent PSUM across all chunks; the denominator
    comes from E.T @ ones into the same PSUM bank at a different partition
    quadrant.  The final per-utterance MLP runs once at the end in bf16.
  - The whole program is emitted statically as a 6-stage software pipeline
    over chunks (L1 / L2 / L3 / L4+score-product / score-tail / pooling,
    each one iteration apart), so every matmul -> relu -> next-layer
    dependency crosses a full ~19us iteration and the PE never waits on the
    other engines; this also keeps the PE out of its low p-states.  Relu
    drains alternate between Scalar and DVE per psum tile so each layer
    phase drains as fast as the PE fills it.
"""

import sys

sys.path.insert(0, "/opt/trn_rl_repo")

import numpy as np
import ml_dtypes

import concourse.bass as bass
import concourse.mybir as mybir
import concourse.tile as tile
from concourse import bacc
from concourse.bass_utils import run_bass_kernel_spmd

P = 128
FEAT = 78
HID = 1024
NCLS = 10
NSEG = 256
NCORES = 8
SEGS_PER_CORE = NSEG // NCORES
CH = 512           # frames per chunk (free dim of the layer matmuls)
FRT_PER_CH = CH // P
KS = HID // P      # 8 k-subtiles
F32 = mybir.dt.float32
F32R = mybir.dt.float32r
BF16 = mybir.dt.bfloat16
F8 = mybir.dt.float8e4
DR = mybir.MatmulPerfMode.DoubleRow
E4NP = ml_dtypes.float8_e4m3
BFNP = ml_dtypes.bfloat16

# misc constant tile column layout ([128, 32] f32, host-packed)
MC_B2 = 0          # cols 0..7   : b2 striped [128, 8]
MC_B3 = 8          # cols 8..15  : b3 striped
MC_B5 = 17         # col 17      : b5 replicated down partitions
# bf16 const tile ([128, 96])
CB_ONES8 = 0       # cols 0..7  : ones (denom matmul rhs)
CB_ID = 8          # cols 8..39, rows 0..31: 32x32 identity
CB_W7 = 40         # cols 40..119?? keep within 96: W7 as [128, 8, 10] -> 80 cols
# fp8 const row ([1, 2, 1536]): ones pair + b4 (hi, lo) pair
# row layout [1, 2, 1536]: [:, :, 0:128] ones, [:, :, 512:1536] b4 hi/lo
# simpler: two fields side by side, see prepare_inputs
# bf16 row consts ([1, 64])
RW_B7 = 0          # cols 0..9 : b7
RW_ONES = 16       # cols 16..48 : ones row (bias matmuls, final MLP)


def _segment_ids(lengths: np.ndarray, total: int) -> np.ndarray:
    """Replicate jnp.repeat(arange(n), lengths, total_repeat_length=total)."""
    lengths = np.asarray(lengths, dtype=np.int64)
    seg = np.repeat(np.arange(lengths.shape[0], dtype=np.int32), np.maximum(lengths, 0))
    if seg.shape[0] >= total:
        return seg[:total]
    pad_val = seg[-1] if seg.shape[0] > 0 else np.int32(0)
    return np.concatenate([seg, np.full(total - seg.shape[0], pad_val, np.int32)])


def _balance_segments(lengths: np.ndarray) -> list[list[int]]:
    """Assign 256 segments to 8 cores, 32 each, minimizing max frame count."""
    order = np.argsort(-lengths, kind="stable")
    loads = [0] * NCORES
    bins: list[list[int]] = [[] for _ in range(NCORES)]
    for s in order:
        cands = [c for c in range(NCORES) if len(bins[c]) < SEGS_PER_CORE]
        c = min(cands, key=lambda c: (loads[c], c))
        bins[c].append(int(s))
        loads[c] += int(lengths[s])
    for b in bins:
        b.sort()
    return bins


def _build_program(m_pad: int):
    """Emit the Bass/Tile program for one core with m_pad frames (static)."""
    nch = m_pad // CH
    frt = m_pad // P
    S = SEGS_PER_CORE

    nc = bacc.Bacc("TRN2", target_bir_lowering=False, debug=False,
                   num_devices=NCORES)

    xT_d = nc.dram_tensor("xT", [P, m_pad], BF16, kind="ExternalInput")
    A_d = nc.dram_tensor("Amat", [P, frt, S], BF16, kind="ExternalInput")
    W1_d = nc.dram_tensor("W1p", [P, HID], BF16, kind="ExternalInput")
    W2_d = nc.dram_tensor("W2q", [P, KS, HID], F8, kind="ExternalInput")
    W3_d = nc.dram_tensor("W3q", [P, KS, HID], F8, kind="ExternalInput")
    W4_d = nc.dram_tensor("W4q", [P, KS, HID], F8, kind="ExternalInput")
    W5_d = nc.dram_tensor("W5rep", [P, HID], BF16, kind="ExternalInput")
    W6_d = nc.dram_tensor("W6b", [P, KS, HID], BF16, kind="ExternalInput")
    b6_d = nc.dram_tensor("b6r", [1, HID], BF16, kind="ExternalInput")
    misc_d = nc.dram_tensor("miscc", [P, 32], F32, kind="ExternalInput")
    cbf_d = nc.dram_tensor("cbf", [P, 128], BF16, kind="ExternalInput")
    c8_d = nc.dram_tensor("c8", [1, 2, 1536], F8, kind="ExternalInput")
    rw_d = nc.dram_tensor("rwb", [1, 64], BF16, kind="ExternalInput")
    out_d = nc.dram_tensor("out", [S, NCLS], F32, kind="ExternalOutput")

    RELU = mybir.ActivationFunctionType.Relu
    EXP = mybir.ActivationFunctionType.Exp
    MULT = mybir.AluOpType.mult
    ADD = mybir.AluOpType.add
    MAX = mybir.AluOpType.max

    with tile.TileContext(nc) as tc:
        with (
            tc.tile_pool(name="wpool", bufs=1) as wpool,
            tc.tile_pool(name="xpool", bufs=6) as xpool,
            tc.tile_pool(name="apool", bufs=10) as apool,
            tc.tile_pool(name="h1pool", bufs=2) as h1pool,
            tc.tile_pool(name="h2pool", bufs=2) as h2pool,
            tc.tile_pool(name="h3pool", bufs=2) as h3pool,
            tc.tile_pool(name="h4pool", bufs=4) as h4pool,
            tc.tile_pool(name="scrpool", bufs=4) as scrpool,
            tc.tile_pool(name="colpool", bufs=4) as colpool,
            tc.tile_pool(name="epool", bufs=4) as epool,
            tc.tile_pool(name="fpool", bufs=1) as fpool,
            tc.tile_pool(name="psL", bufs=7, space="PSUM") as psL,
            tc.tile_pool(name="psAcc", bufs=1, space="PSUM") as psAcc,
        ):
            # ---- resident constants/weights ----
            W1s = wpool.tile([P, HID], BF16, tag="W1")
            nc.sync.dma_start(W1s[:], W1_d.ap())

            def load_w(d, tagp, dt):
                t = wpool.tile([P, KS, HID], dt, tag=tagp)
                for k in range(KS):
                    nc.sync.dma_start(t[:, k, :], d.ap()[:, k, :])
                return t

            misc = wpool.tile([P, 32], F32, tag="misc")
            nc.sync.dma_start(misc[:], misc_d.ap())
            cbf = wpool.tile([P, 128], BF16, tag="cbf")
            nc.sync.dma_start(cbf[:], cbf_d.ap())
            c8 = wpool.tile([1, 2, 1536], F8, tag="c8")
            nc.sync.dma_start(c8[:], c8_d.ap())
            rwb = wpool.tile([1, 64], BF16, tag="rwb")
            nc.sync.dma_start(rwb[:], rw_d.ap())

            # prefetch the first six chunks' x/A BEFORE the big weight
            # loads: L1 of chunk 0 only needs W1 + x(0), so the PE can
            # start ~2us in instead of waiting ~21us for all weights.
            pre_ = {}
            for c0 in range(min(6, nch)):
                xt0 = xpool.tile([P, CH], BF16, tag="x")
                nc.sync.dma_start(xt0[:], xT_d.ap()[:, c0 * CH:(c0 + 1) * CH])
                ag0 = apool.tile([P, FRT_PER_CH, S], BF16, tag="A")
                nc.sync.dma_start(
                    ag0[:], A_d.ap()[:, c0 * FRT_PER_CH:
                                     (c0 + 1) * FRT_PER_CH, :])
                pre_[c0] = {"ag": ag0, "xt": xt0}

            W2s = load_w(W2_d, "W2q", F8)
            W3s = load_w(W3_d, "W3q", F8)
            W4s = load_w(W4_d, "W4q", F8)
            W5s4 = wpool.tile([P, FRT_PER_CH, HID], BF16, tag="W5")
            for f in range(FRT_PER_CH):
                nc.sync.dma_start(W5s4[:, f, :], W5_d.ap())

            b5col = misc[:, MC_B5:MC_B5 + 1]
            ones8 = cbf[:, CB_ONES8:CB_ONES8 + 8]
            ident = cbf[:S, CB_ID:CB_ID + S]
            W7v = cbf[:, CB_W7:CB_W7 + KS * NCLS].rearrange(
                "p (o c) -> p o c", c=NCLS)
            b7row = rwb[:, RW_B7:RW_B7 + NCLS]
            ones_row = rwb[:, RW_ONES:RW_ONES + S]
            ones_pair8 = c8[:, :, 0:P]          # [1, 2, 128] of ones (fp8)
            b4pair = c8[:, :, 512:1536]         # [1, 2, 1024] b4 (hi, lo)

            # persistent PSUM accumulators, all in ONE bank at different
            # partition quadrants (frees a bank for deeper psum rotation):
            #   pooled0 [0:32, 0:512], pooled1 [32:64, 0:512],
            #   denom [64:96, 0:8]
            acc0 = psAcc.tile([P, 512], F32, tag="acc0")
            pooled0 = acc0[0:S, :]
            pooled1 = acc0[S:2 * S, :]
            denom = acc0[2 * S:3 * S, 0:8]

            # ---- main pass: 5-stage software pipeline over chunks ----
            # Stage k of chunk c runs in iteration c+k, so every cross-engine
            # dependency (matmul -> relu -> next layer's matmul) has a full
            # iteration (~15us) of slack and the PE never waits on the relus.
            st_ = {}   # per-chunk tile state

            def relu_ps(out, in_, bias, eng):
                """relu(in_ + bias) -> out (fp8/bf16 cast) on Scalar or DVE."""
                if eng == 0:
                    nc.scalar.activation(out, in_, RELU,
                                         bias=0.0 if bias is None else bias)
                elif bias is None:
                    nc.vector.tensor_scalar_max(out, in_, 0.0)
                else:
                    nc.vector.tensor_scalar(out=out, in0=in_, scalar1=bias,
                                            scalar2=0.0, op0=ADD, op1=MAX)

            def s0(c):  # prefetch x/A one iteration ahead of the L1 use
                if c in pre_:
                    st_[c] = pre_[c]
                    return
                xt = xpool.tile([P, CH], BF16, tag="x")
                nc.sync.dma_start(xt[:], xT_d.ap()[:, c * CH:(c + 1) * CH])
                ag = apool.tile([P, FRT_PER_CH, S], BF16, tag="A")
                nc.sync.dma_start(
                    ag[:], A_d.ap()[:, c * FRT_PER_CH:(c + 1) * FRT_PER_CH, :])
                st_[c] = {"ag": ag, "xt": xt}

            def s1(c):  # L1 (bf16) -> h1 fp8 (per-m relu, alternating eng)
                xt = st_[c]["xt"]
                h1 = h1pool.tile([P, KS, CH], F8, tag="h1")
                for m in range(KS):
                    ps = psL.tile([P, CH], F32, tag="mm")
                    nc.tensor.matmul(ps[:], W1s[:, m * P:(m + 1) * P], xt[:],
                                     start=True, stop=True)
                    relu_ps(h1[:, m, :], ps[:], None,
                            0 if m in (0, 2, 4, 6, 7) else 1)
                st_[c]["h1"] = h1

            def _mid_layer(c, Ws, hin_key, hout_key, pool, boff, flip):
                """L2/L3: fp8 DoubleRow + per-m relu(+bias), alternating
                engines per psum-tile half so the drain keeps up with PE."""
                hin = st_[c][hin_key]
                hout = pool.tile([P, KS, CH], F8, tag=hout_key)
                for m in range(KS):
                    ps = psL.tile([P, CH], F32, tag="mm")
                    for t in range(KS // 2):
                        nc.tensor.matmul(
                            ps[:],
                            Ws[:, 2 * t:2 * t + 2, m * P:(m + 1) * P],
                            hin[:, 2 * t:2 * t + 2, :],
                            start=(t == 0), stop=(t == KS // 2 - 1),
                            perf_mode=DR)
                    relu_ps(hout[:, m, :], ps[:],
                            misc[:, boff + m:boff + m + 1], (m + flip) % 2)
                st_[c][hout_key] = hout

            def s2(c):
                _mid_layer(c, W2s, "h1", "h2", h2pool, MC_B2, 0)

            def s3(c):
                _mid_layer(c, W3s, "h2", "h3", h3pool, MC_B3, 1)

            def s4(c):  # L4 fp8 DoubleRow -> h4 bf16; scores -> E (bf16)
                h3 = st_[c]["h3"]
                ag = st_[c]["ag"]
                h4 = h4pool.tile([P, FRT_PER_CH, HID], BF16, tag="h4")
                for f in range(FRT_PER_CH):
                    for n in range(2):
                        ps4 = psL.tile([P, CH], F32, tag="mm")
                        nc.tensor.matmul(ps4[:], ones_pair8,
                                         b4pair[:, :, n * 512:(n + 1) * 512],
                                         start=True, stop=False, perf_mode=DR)
                        for t in range(KS // 2):
                            nc.tensor.matmul(
                                ps4[:],
                                h3[:, 2 * t:2 * t + 2, f * P:(f + 1) * P],
                                W4s[:, 2 * t:2 * t + 2, n * 512:(n + 1) * 512],
                                start=False, stop=(t == KS // 2 - 1),
                                perf_mode=DR)
                        relu_ps(h4[:, f, n * 512:(n + 1) * 512], ps4[:], None,
                                0 if (2 * f + n) in (0, 2, 4, 6, 7) else 1)
                # scores, fully batched: one GpSimd product over all four
                # f-tiles, one DVE reduce (innermost axis), one exp, one
                # clamp; per-f E columns on GpSimd.  E is consumed by the
                # pooling matmuls one iteration later.
                scr = scrpool.tile([P, FRT_PER_CH, HID], BF16, tag="scr")
                for f in range(FRT_PER_CH):
                    nc.gpsimd.tensor_mul(scr[:, f, :], h4[:, f, :],
                                         W5s4[:, f, :])
                st_[c]["h4"] = h4
                st_[c]["scr"] = scr

            def s4b(c):  # score reduces at the very END of the DVE stream
                # (their products are an iteration old, so they never
                # head-of-line-block the DVE act drains).
                scr = st_[c]["scr"]
                ct = colpool.tile([P, FRT_PER_CH], F32, tag="ctb")
                for f in range(FRT_PER_CH):
                    nc.vector.tensor_reduce(out=ct[:, f:f + 1],
                                            in_=scr[:, f, :],
                                            axis=mybir.AxisListType.X, op=ADD)
                st_[c]["ct"] = ct

            def s4c(c):  # exp/clamp/E at the START of the next iteration:
                # exp leads the Scalar queue (its reduces are an iteration
                # old) so Pool's clamp/E never wait deep into the iteration
                # and Pool's next product batch starts on time.
                ag = st_[c]["ag"]
                ct = st_[c]["ct"]
                etg = epool.tile([P, FRT_PER_CH, S], BF16, tag="E")
                ec = colpool.tile([P, 2 * FRT_PER_CH], F32, tag="ec")
                nc.scalar.activation(ec[:, :FRT_PER_CH], ct[:], EXP,
                                     bias=b5col)
                nc.gpsimd.tensor_scalar_max(ec[:, FRT_PER_CH:],
                                            ec[:, :FRT_PER_CH], 1.0)
                for f in range(FRT_PER_CH):
                    nc.gpsimd.tensor_scalar_mul(
                        etg[:, f, :], ag[:, f, :],
                        ec[:, FRT_PER_CH + f:FRT_PER_CH + f + 1])
                st_[c]["et"] = etg

            def s5(c):  # pooling matmuls (persistent PSUM accumulation)
                h4 = st_[c]["h4"]
                etg = st_[c]["et"]
                first = c == 0
                last = c == nch - 1
                for f in range(FRT_PER_CH):
                    et = etg[:, f, :]
                    st = bool(first and f == 0)
                    sp = bool(last and f == FRT_PER_CH - 1)
                    # pooled0/denom share a PSUM bank at different partition
                    # quadrants; the sim's group check is partition-blind so
                    # it must be skipped (values verified exact in CoreSim).
                    nc.tensor.matmul(pooled0, et, h4[:, f, :512],
                                     start=st, stop=sp, skip_group_check=True)
                    nc.tensor.matmul(pooled1, et, h4[:, f, 512:],
                                     start=st, stop=sp, skip_group_check=True)
                    nc.tensor.matmul(denom, et, ones8,
                                     start=st, stop=sp, skip_group_check=True)
                del st_[c]

            sched = ((s4c, 5), (s0, -1), (s1, 0), (s2, 1), (s3, 2),
                     (s4, 3), (s4b, 4), (s5, 5))
            for i in range(-1, nch + 5):
                for stage, off in sched:
                    c = i - off
                    if 0 <= c < nch:
                        stage(c)

            # ---- final per-utterance MLP ----
            W6s = load_w(W6_d, "W6b", BF16)
            b6s = wpool.tile([1, HID], BF16, tag="b6")
            nc.sync.dma_start(b6s[:], b6_d.ap())

            # 1/denom: copy the [64:96] psum quadrant to SBUF, DMA-shift it
            # to partitions 0:32 and 32:64 (engines can't move across lanes)
            dtmp = fpool.tile([3 * S, 1], F32, tag="dtmp")
            nc.vector.tensor_copy(out=dtmp[2 * S:3 * S, 0:1],
                                  in_=denom[:, 0:1])
            fc = colpool.tile([2 * S, 4], F32, tag="col")
            nc.sync.dma_start(fc[0:S, 0:1], dtmp[2 * S:3 * S, 0:1])
            nc.sync.dma_start(fc[S:2 * S, 0:1], dtmp[2 * S:3 * S, 0:1])
            nc.vector.reciprocal(fc[:, 1:2], fc[:, 0:1])

            # pooled (normalized) in f32 for the PE transpose; pooled1 is
            # scaled in place at partitions 32:64, then DMA-shifted down
            pooled_sb = fpool.tile([S, HID], F32, tag="pooled")
            pstg = fpool.tile([2 * S, 512], F32, tag="pstg")
            nc.vector.tensor_scalar_mul(pooled_sb[:, :512], pooled0,
                                        fc[0:S, 1:2])
            nc.vector.tensor_scalar_mul(pstg[S:2 * S, :], pooled1,
                                        fc[S:2 * S, 1:2])
            nc.sync.dma_start(pooled_sb[:, 512:], pstg[S:2 * S, :])

            # transpose pooled -> pooledT [hid, seg] (bf16 via cast copies)
            identf = fpool.tile([S, S], F32, tag="identf")
            nc.vector.tensor_copy(out=identf[:], in_=ident)
            tposed = fpool.tile([P, KS, 2 * S], BF16, tag="tposed")
            pooledT = tposed[:, :, :S]
            gT = tposed[:, :, S:]
            for k in range(KS):
                pst = psL.tile([P, CH], F32, tag="mm")
                nc.tensor.transpose(pst[:, :S], pooled_sb[:, k * P:(k + 1) * P],
                                    identf[:])
                nc.vector.tensor_copy(out=pooledT[:, k, :], in_=pst[:, :S])

            # g = relu(pooled @ W6 + b6)   (seg-major [S, HID], bf16)
            g_sb = fpool.tile([S, HID], BF16, tag="g")
            for n in range(2):
                psg = psL.tile([P, CH], F32, tag="mm")
                for k in range(KS):
                    nc.tensor.matmul(psg[:S, :], pooledT[:, k, :],
                                     W6s[:, k, n * 512:(n + 1) * 512],
                                     start=(k == 0), stop=False)
                nc.tensor.matmul(psg[:S, :], ones_row,
                                 b6s[:, n * 512:(n + 1) * 512],
                                 start=False, stop=True)
                nc.scalar.activation(g_sb[:, n * 512:(n + 1) * 512],
                                     psg[:S, :], RELU)

            # gT [hid, seg] (transpose back via f32 staging)
            gf = fpool.tile([S, HID], F32, tag="gf")
            nc.vector.tensor_copy(out=gf[:], in_=g_sb[:])
            for k in range(KS):
                pst = psL.tile([P, CH], F32, tag="mm")
                nc.tensor.transpose(pst[:, :S], gf[:, k * P:(k + 1) * P],
                                    identf[:])
                nc.vector.tensor_copy(out=gT[:, k, :], in_=pst[:, :S])

            # out = g @ W7 + b7
            pso = psL.tile([P, CH], F32, tag="mm")
            for k in range(KS):
                nc.tensor.matmul(pso[:S, :NCLS], gT[:, k, :], W7v[:, k, :],
                                 start=(k == 0), stop=False)
            nc.tensor.matmul(pso[:S, :NCLS], ones_row, b7row,
                             start=False, stop=True)
            oc = colpool.tile([S, 16], F32, tag="oc")
            nc.vector.tensor_copy(out=oc[:, :NCLS], in_=pso[:S, :NCLS])
            nc.sync.dma_start(out_d.ap()[:], oc[:, :NCLS])

    nc.compile()
    return nc


def prepare_inputs(x, W1, b1, W2, b2, W3, b3, W4, b4, W5, b5, W6, b6, W7, b7,
                   lengths):
    """Host-side sharding/packing. Returns (in_maps, bins, m_pad)."""
    x = np.ascontiguousarray(np.asarray(x, dtype=np.float32))
    lengths = np.asarray(lengths)
    total = x.shape[0]
    seg_ids = _segment_ids(lengths, total)
    counts = np.bincount(seg_ids, minlength=NSEG).astype(np.int64)
    starts = np.zeros(NSEG + 1, dtype=np.int64)
    starts[1:] = np.cumsum(counts)

    bins = _balance_segments(counts)
    core_frames = [int(sum(counts[s] for s in b)) for b in bins]
    m_pad = ((max(core_frames) + CH - 1) // CH) * CH
    frt = m_pad // P

    W1p = np.zeros((P, HID), dtype=np.float32)
    W1p[:FEAT] = np.asarray(W1, dtype=np.float32)
    W1p[FEAT] = np.asarray(b1, dtype=np.float32)

    def dr_pack(W, dt):
        """[1024, 1024] -> [128, 8, 1024] with Wq[p, k, m] = W[k*128+p, m]."""
        Wf = np.asarray(W, np.float32).reshape(KS, P, HID)
        return np.ascontiguousarray(Wf.transpose(1, 0, 2)).astype(dt)

    misc = np.zeros((P, 32), dtype=np.float32)
    misc[:, MC_B2:MC_B2 + KS] = np.asarray(b2, np.float32).reshape(KS, P).T
    misc[:, MC_B3:MC_B3 + KS] = np.asarray(b3, np.float32).reshape(KS, P).T
    misc[:, MC_B5] = np.float32(np.asarray(b5, np.float32).reshape(-1)[0])

    cbf = np.zeros((P, 128), dtype=np.float32)
    cbf[:, CB_ONES8:CB_ONES8 + 8] = 1.0
    cbf[:SEGS_PER_CORE, CB_ID:CB_ID + SEGS_PER_CORE] = np.eye(
        SEGS_PER_CORE, dtype=np.float32)
    cbf[:, CB_W7:CB_W7 + KS * NCLS] = np.asarray(W7, np.float32).reshape(
        KS, P, NCLS).transpose(1, 0, 2).reshape(P, KS * NCLS)

    rwb = np.zeros((1, 64), dtype=np.float32)
    rwb[0, RW_B7:RW_B7 + NCLS] = np.asarray(b7, np.float32).reshape(-1)
    rwb[0, RW_ONES:RW_ONES + SEGS_PER_CORE] = 1.0

    c8 = np.zeros((1, 2, 1536), dtype=np.float32)
    c8[0, :, 0:P] = 1.0
    b4f = np.asarray(b4, np.float32).reshape(-1)
    b4hi = b4f.astype(E4NP).astype(np.float32)
    b4lo = (b4f - b4hi).astype(E4NP).astype(np.float32)
    c8[0, 0, 512:1536] = b4hi
    c8[0, 1, 512:1536] = b4lo

    shared = dict(
        W1p=W1p.astype(BFNP),
        W2q=dr_pack(W2, E4NP),
        W3q=dr_pack(W3, E4NP),
        W4q=dr_pack(W4, E4NP),
        W5rep=np.broadcast_to(np.asarray(W5, np.float32).reshape(1, HID),
                              (P, HID)).astype(BFNP),
        W6b=dr_pack(W6, BFNP),
        b6r=np.asarray(b6, np.float32).reshape(1, HID).astype(BFNP),
        miscc=misc,
        cbf=cbf.astype(BFNP),
        c8=c8.astype(E4NP),
        rwb=rwb.astype(BFNP),
    )

    in_maps = []
    for core in range(NCORES):
        segs = bins[core]
        xs = [x[starts[s]:starts[s + 1]] for s in segs]
        xcat = np.concatenate(xs, axis=0) if xs else np.zeros((0, FEAT), np.float32)
        n = xcat.shape[0]
        xT = np.zeros((P, m_pad), dtype=np.float32)
        xT[:FEAT, :n] = xcat.T
        xT[FEAT, :n] = 1.0  # constant feature -> b1
        A = np.zeros((m_pad, SEGS_PER_CORE), dtype=np.float32)
        off = 0
        for j, s in enumerate(segs):
            ln = int(counts[s])
            A[off:off + ln, j] = 1.0
            off += ln
        im = dict(shared)
        im["xT"] = xT.astype(BFNP)
        # partition-major layout [P, frt, S]: Ah[p, t, s] = A[t*128 + p, s]
        im["Amat"] = np.ascontiguousarray(
            A.reshape(frt, P, SEGS_PER_CORE).transpose(1, 0, 2)).astype(BFNP)
        in_maps.append(im)
    return in_maps, bins, m_pad


_PROGRAM_CACHE: dict[int, object] = {}


def kernel(**inputs) -> np.ndarray:
    in_maps, bins, m_pad = prepare_inputs(**inputs)
    nc = _PROGRAM_CACHE.get(m_pad)
    if nc is None:
        nc = _build_program(m_pad)
        _PROGRAM_CACHE[m_pad] = nc
    res = run_bass_kernel_spmd(nc, in_maps, core_ids=list(range(NCORES)))
    out = np.zeros((NSEG, NCLS), dtype=np.float32)
    for core in range(NCORES):
        out[bins[core]] = res.results[core]["out"]
    return out


# revision 38
# speedup vs baseline: 1.0085x; 1.0085x over previous
"""Trainium2 Bass kernel for nn_Dnn_with_Attention (ragged attention-pooled DNN).

Contract: kernel(**inputs) takes FULL unsharded numpy inputs (keys as in
reference.setup_inputs()) and returns the FULL [256, 10] float32 output.

Strategy (data-parallel over utterances, 8 NeuronCores):
  - Host: greedily balance the 256 segments over 8 cores (32 whole segments
    each), gather each core's frames, transpose x to feature-major
    bf16 [128(feat-padded), M_PAD] and build a per-frame one-hot segment
    membership matrix A (bf16).  A row of ones is appended as feature 78 so
    b1 folds into W1.
  - Device (per core): L1 in bf16 (feature-major, [1024, frames]); L2/L3/L4
    run in fp8 e4m3 with MatmulPerfMode.DoubleRow (two 128-K slices per
    instruction at 0.5 cycles/row, ~4x the f32r rate).  Weights W2/W3/W4 are
    host-quantized to e4m3; inter-layer activations are written as e4m3
    directly by the relu ops.  L4 produces frame-major h4 in bf16; b4 is
    added via a DoubleRow matmul against a host-packed (hi, lo) e4m3 pair so
    the quantization error cancels.  Scores: GpSimd computes h4 * W5 (SBUF
    only; GPSIMD cannot touch PSUM), DVE reduces the innermost axis, Scalar
    takes the exp; e = max(exp(score + b5), 1) folds the relu.  Segment
    softmax pooling is small PE matmuls E.T @ h4 (E = A * e, bf16)
    accumulated into persistent PSUM across all chunks; the denominator
    comes from E.T @ ones into the same PSUM bank at a different partition
    quadrant.  The final per-utterance MLP runs once at the end in bf16.
  - The whole program is emitted statically as a 6-stage software pipeline
    over chunks (L1 / L2 / L3 / L4+score-product / score-tail / pooling,
    each one iteration apart), so every matmul -> relu -> next-layer
    dependency crosses a full ~19us iteration and the PE never waits on the
    other engines; this also keeps the PE out of its low p-states.  Relu
    drains alternate between Scalar and DVE per psum tile so each layer
    phase drains as fast as the PE fills it.
"""

import sys

sys.path.insert(0, "/opt/trn_rl_repo")

import numpy as np
import ml_dtypes

import concourse.bass as bass
import concourse.mybir as mybir
import concourse.tile as tile
from concourse import bacc
from concourse.bass_utils import run_bass_kernel_spmd

P = 128
FEAT = 78
HID = 1024
NCLS = 10
NSEG = 256
NCORES = 8
SEGS_PER_CORE = NSEG // NCORES
CH = 512           # frames per chunk (free dim of the layer matmuls)
FRT_PER_CH = CH // P
KS = HID // P      # 8 k-subtiles
F32 = mybir.dt.float32
F32R = mybir.dt.float32r
BF16 = mybir.dt.bfloat16
F8 = mybir.dt.float8e4
DR = mybir.MatmulPerfMode.DoubleRow
E4NP = ml_dtypes.float8_e4m3
BFNP = ml_dtypes.bfloat16

# misc constant tile column layout ([128, 32] f32, host-packed)
MC_B2 = 0          # cols 0..7   : b2 striped [128, 8]
MC_B3 = 8          # cols 8..15  : b3 striped
MC_B5 = 17         # col 17      : b5 replicated down partitions
# bf16 const tile ([128, 96])
CB_ONES8 = 0       # cols 0..7  : ones (denom matmul rhs)
CB_ID = 8          # cols 8..39, rows 0..31: 32x32 identity
CB_W7 = 40         # cols 40..119?? keep within 96: W7 as [128, 8, 10] -> 80 cols
# fp8 const row ([1, 2, 1536]): ones pair + b4 (hi, lo) pair
# row layout [1, 2, 1536]: [:, :, 0:128] ones, [:, :, 512:1536] b4 hi/lo
# simpler: two fields side by side, see prepare_inputs
# bf16 row consts ([1, 64])
RW_B7 = 0          # cols 0..9 : b7
RW_ONES = 16       # cols 16..48 : ones row (bias matmuls, final MLP)


def _segment_ids(lengths: np.ndarray, total: int) -> np.ndarray:
    """Replicate jnp.repeat(arange(n), lengths, total_repeat_length=total)."""
    lengths = np.asarray(lengths, dtype=np.int64)
    seg = np.repeat(np.arange(lengths.shape[0], dtype=np.int32), np.maximum(lengths, 0))
    if seg.shape[0] >= total:
        return seg[:total]
    pad_val = seg[-1] if seg.shape[0] > 0 else np.int32(0)
    return np.concatenate([seg, np.full(total - seg.shape[0], pad_val, np.int32)])


def _balance_segments(lengths: np.ndarray) -> list[list[int]]:
    """Assign 256 segments to 8 cores, 32 each, minimizing max frame count."""
    order = np.argsort(-lengths, kind="stable")
    loads = [0] * NCORES
    bins: list[list[int]] = [[] for _ in range(NCORES)]
    for s in order:
        cands = [c for c in range(NCORES) if len(bins[c]) < SEGS_PER_CORE]
        c = min(cands, key=lambda c: (loads[c], c))
        bins[c].append(int(s))
        loads[c] += int(lengths[s])
    for b in bins:
        b.sort()
    return bins


def _build_program(m_pad: int):
    """Emit the Bass/Tile program for one core with m_pad frames (static)."""
    nch = m_pad // CH
    frt = m_pad // P
    S = SEGS_PER_CORE

    nc = bacc.Bacc("TRN2", target_bir_lowering=False, debug=False,
                   num_devices=NCORES)

    xT_d = nc.dram_tensor("xT", [P, m_pad], BF16, kind="ExternalInput")
    A_d = nc.dram_tensor("Amat", [P, frt, S], BF16, kind="ExternalInput")
    W1_d = nc.dram_tensor("W1p", [P, HID], BF16, kind="ExternalInput")
    W2_d = nc.dram_tensor("W2q", [P, KS, HID], F8, kind="ExternalInput")
    W3_d = nc.dram_tensor("W3q", [P, KS, HID], F8, kind="ExternalInput")
    W4_d = nc.dram_tensor("W4q", [P, KS, HID], F8, kind="ExternalInput")
    W5_d = nc.dram_tensor("W5rep", [P, HID], BF16, kind="ExternalInput")
    W6_d = nc.dram_tensor("W6b", [P, KS, HID], BF16, kind="ExternalInput")
    b6_d = nc.dram_tensor("b6r", [1, HID], BF16, kind="ExternalInput")
    misc_d = nc.dram_tensor("miscc", [P, 32], F32, kind="ExternalInput")
    cbf_d = nc.dram_tensor("cbf", [P, 128], BF16, kind="ExternalInput")
    c8_d = nc.dram_tensor("c8", [1, 2, 1536], F8, kind="ExternalInput")
    rw_d = nc.dram_tensor("rwb", [1, 64], BF16, kind="ExternalInput")
    out_d = nc.dram_tensor("out", [S, NCLS], F32, kind="ExternalOutput")

    RELU = mybir.ActivationFunctionType.Relu
    EXP = mybir.ActivationFunctionType.Exp
    MULT = mybir.AluOpType.mult
    ADD = mybir.AluOpType.add
    MAX = mybir.AluOpType.max

    with tile.TileContext(nc) as tc:
        with (
            tc.tile_pool(name="wpool", bufs=1) as wpool,
            tc.tile_pool(name="xpool", bufs=6) as xpool,
            tc.tile_pool(name="apool", bufs=10) as apool,
            tc.tile_pool(name="h1pool", bufs=2) as h1pool,
            tc.tile_pool(name="h2pool", bufs=2) as h2pool,
            tc.tile_pool(name="h3pool", bufs=2) as h3pool,
            tc.tile_pool(name="h4pool", bufs=4) as h4pool,
            tc.tile_pool(name="scrpool", bufs=4) as scrpool,
            tc.tile_pool(name="colpool", bufs=4) as colpool,
            tc.tile_pool(name="epool", bufs=4) as epool,
            tc.tile_pool(name="fpool", bufs=1) as fpool,
            tc.tile_pool(name="psL", bufs=7, space="PSUM") as psL,
            tc.tile_pool(name="psAcc", bufs=1, space="PSUM") as psAcc,
        ):
            # ---- resident constants/weights ----
            W1s = wpool.tile([P, HID], BF16, tag="W1")
            nc.sync.dma_start(W1s[:], W1_d.ap())

            def load_w(d, tagp, dt):
                t = wpool.tile([P, KS, HID], dt, tag=tagp)
                for k in range(KS):
                    nc.sync.dma_start(t[:, k, :], d.ap()[:, k, :])
                return t

            misc = wpool.tile([P, 32], F32, tag="misc")
            nc.sync.dma_start(misc[:], misc_d.ap())
            cbf = wpool.tile([P, 128], BF16, tag="cbf")
            nc.sync.dma_start(cbf[:], cbf_d.ap())
            c8 = wpool.tile([1, 2, 1536], F8, tag="c8")
            nc.sync.dma_start(c8[:], c8_d.ap())
            rwb = wpool.tile([1, 64], BF16, tag="rwb")
            nc.sync.dma_start(rwb[:], rw_d.ap())

            # prefetch the first six chunks' x/A BEFORE the big weight
            # loads: L1 of chunk 0 only needs W1 + x(0), so the PE can
            # start ~2us in instead of waiting ~21us for all weights.
            pre_ = {}
            for c0 in range(min(6, nch)):
                xt0 = xpool.tile([P, CH], BF16, tag="x")
                nc.sync.dma_start(xt0[:], xT_d.ap()[:, c0 * CH:(c0 + 1) * CH])
                ag0 = apool.tile([P, FRT_PER_CH, S], BF16, tag="A")
                nc.sync.dma_start(
                    ag0[:], A_d.ap()[:, c0 * FRT_PER_CH:
                                     (c0 + 1) * FRT_PER_CH, :])
                pre_[c0] = {"ag": ag0, "xt": xt0}

            W2s = load_w(W2_d, "W2q", F8)
            W3s = load_w(W3_d, "W3q", F8)
            W4s = load_w(W4_d, "W4q", F8)
            W5s4 = wpool.tile([P, FRT_PER_CH, HID], BF16, tag="W5")
            for f in range(FRT_PER_CH):
                nc.sync.dma_start(W5s4[:, f, :], W5_d.ap())

            b5col = misc[:, MC_B5:MC_B5 + 1]
            ones8 = cbf[:, CB_ONES8:CB_ONES8 + 8]
            ident = cbf[:S, CB_ID:CB_ID + S]
            W7v = cbf[:, CB_W7:CB_W7 + KS * NCLS].rearrange(
                "p (o c) -> p o c", c=NCLS)
            b7row = rwb[:, RW_B7:RW_B7 + NCLS]
            ones_row = rwb[:, RW_ONES:RW_ONES + S]
            ones_pair8 = c8[:, :, 0:P]          # [1, 2, 128] of ones (fp8)
            b4pair = c8[:, :, 512:1536]         # [1, 2, 1024] b4 (hi, lo)

            # persistent PSUM accumulators, all in ONE bank at different
            # partition quadrants (frees a bank for deeper psum rotation):
            #   pooled0 [0:32, 0:512], pooled1 [32:64, 0:512],
            #   denom [64:96, 0:8]
            acc0 = psAcc.tile([P, 512], F32, tag="acc0")
            pooled0 = acc0[0:S, :]
            pooled1 = acc0[S:2 * S, :]
            denom = acc0[2 * S:3 * S, 0:8]

            # ---- main pass: 5-stage software pipeline over chunks ----
            # Stage k of chunk c runs in iteration c+k, so every cross-engine
            # dependency (matmul -> relu -> next layer's matmul) has a full
            # iteration (~15us) of slack and the PE never waits on the relus.
            st_ = {}   # per-chunk tile state

            def relu_ps(out, in_, bias, eng):
                """relu(in_ + bias) -> out (fp8/bf16 cast) on Scalar or DVE."""
                if eng == 0:
                    nc.scalar.activation(out, in_, RELU,
                                         bias=0.0 if bias is None else bias)
                elif bias is None:
                    nc.vector.tensor_scalar_max(out, in_, 0.0)
                else:
                    nc.vector.tensor_scalar(out=out, in0=in_, scalar1=bias,
                                            scalar2=0.0, op0=ADD, op1=MAX)

            def s0(c):  # prefetch x/A one iteration ahead of the L1 use
                if c in pre_:
                    st_[c] = pre_[c]
                    return
                xt = xpool.tile([P, CH], BF16, tag="x")
                nc.sync.dma_start(xt[:], xT_d.ap()[:, c * CH:(c + 1) * CH])
                ag = apool.tile([P, FRT_PER_CH, S], BF16, tag="A")
                nc.sync.dma_start(
                    ag[:], A_d.ap()[:, c * FRT_PER_CH:(c + 1) * FRT_PER_CH, :])
                st_[c] = {"ag": ag, "xt": xt}

            def s1(c):  # L1 (bf16) -> h1 fp8 (per-m relu, alternating eng)
                xt = st_[c]["xt"]
                h1 = h1pool.tile([P, KS, CH], F8, tag="h1")
                for m in range(KS):
                    ps = psL.tile([P, CH], F32, tag="mm")
                    nc.tensor.matmul(ps[:], W1s[:, m * P:(m + 1) * P], xt[:],
                                     start=True, stop=True)
                    relu_ps(h1[:, m, :], ps[:], None,
                            0 if m in (0, 2, 4, 6, 7) else 1)
                st_[c]["h1"] = h1

            def _mid_layer(c, Ws, hin_key, hout_key, pool, boff, flip):
                """L2/L3: fp8 DoubleRow + per-m relu(+bias), alternating
                engines per psum-tile half so the drain keeps up with PE."""
                hin = st_[c][hin_key]
                hout = pool.tile([P, KS, CH], F8, tag=hout_key)
                for m in range(KS):
                    ps = psL.tile([P, CH], F32, tag="mm")
                    for t in range(KS // 2):
                        nc.tensor.matmul(
                            ps[:],
                            Ws[:, 2 * t:2 * t + 2, m * P:(m + 1) * P],
                            hin[:, 2 * t:2 * t + 2, :],
                            start=(t == 0), stop=(t == KS // 2 - 1),
                            perf_mode=DR)
                    relu_ps(hout[:, m, :], ps[:],
                            misc[:, boff + m:boff + m + 1], (m + flip) % 2)
                st_[c][hout_key] = hout

            def s2(c):
                _mid_layer(c, W2s, "h1", "h2", h2pool, MC_B2, 0)

            def s3(c):
                _mid_layer(c, W3s, "h2", "h3", h3pool, MC_B3, 1)

            def s4(c):  # L4 fp8 DoubleRow -> h4 bf16; scores -> E (bf16)
                h3 = st_[c]["h3"]
                ag = st_[c]["ag"]
                h4 = h4pool.tile([P, FRT_PER_CH, HID], BF16, tag="h4")
                for f in range(FRT_PER_CH):
                    for n in range(2):
                        ps4 = psL.tile([P, CH], F32, tag="mm")
                        nc.tensor.matmul(ps4[:], ones_pair8,
                                         b4pair[:, :, n * 512:(n + 1) * 512],
                                         start=True, stop=False, perf_mode=DR)
                        for t in range(KS // 2):
                            nc.tensor.matmul(
                                ps4[:],
                                h3[:, 2 * t:2 * t + 2, f * P:(f + 1) * P],
                                W4s[:, 2 * t:2 * t + 2, n * 512:(n + 1) * 512],
                                start=False, stop=(t == KS // 2 - 1),
                                perf_mode=DR)
                        relu_ps(h4[:, f, n * 512:(n + 1) * 512], ps4[:], None,
                                0 if (2 * f + n) in (0, 2, 4, 6, 7) else 1)
                # scores, fully batched: one GpSimd product over all four
                # f-tiles, one DVE reduce (innermost axis), one exp, one
                # clamp; per-f E columns on GpSimd.  E is consumed by the
                # pooling matmuls one iteration later.
                scr = scrpool.tile([P, FRT_PER_CH, HID], BF16, tag="scr")
                for f in range(FRT_PER_CH):
                    # drain tail (last two chunks): the product runs on DVE
                    # in the fast bf16 2x mode -- once the main loop ends
                    # nothing hides the serial GpSimd product chain
                    eng = nc.gpsimd if c < nch - 2 else nc.vector
                    eng.tensor_mul(scr[:, f, :], h4[:, f, :], W5s4[:, f, :])
                st_[c]["h4"] = h4
                st_[c]["scr"] = scr

            def s4b(c):  # score reduces at the very END of the DVE stream
                # (their products are an iteration old, so they never
                # head-of-line-block the DVE act drains).
                scr = st_[c]["scr"]
                ct = colpool.tile([P, FRT_PER_CH], F32, tag="ctb")
                for f in range(FRT_PER_CH):
                    nc.vector.tensor_reduce(out=ct[:, f:f + 1],
                                            in_=scr[:, f, :],
                                            axis=mybir.AxisListType.X, op=ADD)
                st_[c]["ct"] = ct

            def s4c(c):  # exp/clamp/E at the START of the next iteration:
                # exp leads the Scalar queue (its reduces are an iteration
                # old) so Pool's clamp/E never wait deep into the iteration
                # and Pool's next product batch starts on time.
                ag = st_[c]["ag"]
                ct = st_[c]["ct"]
                etg = epool.tile([P, FRT_PER_CH, S], BF16, tag="E")
                ec = colpool.tile([P, 2 * FRT_PER_CH], F32, tag="ec")
                nc.scalar.activation(ec[:, :FRT_PER_CH], ct[:], EXP,
                                     bias=b5col)
                nc.gpsimd.tensor_scalar_max(ec[:, FRT_PER_CH:],
                                            ec[:, :FRT_PER_CH], 1.0)
                for f in range(FRT_PER_CH):
                    nc.gpsimd.tensor_scalar_mul(
                        etg[:, f, :], ag[:, f, :],
                        ec[:, FRT_PER_CH + f:FRT_PER_CH + f + 1])
                st_[c]["et"] = etg

            def s5(c):  # pooling matmuls (persistent PSUM accumulation)
                h4 = st_[c]["h4"]
                etg = st_[c]["et"]
                first = c == 0
                last = c == nch - 1
                for f in range(FRT_PER_CH):
                    et = etg[:, f, :]
                    st = bool(first and f == 0)
                    sp = bool(last and f == FRT_PER_CH - 1)
                    # pooled0/denom share a PSUM bank at different partition
                    # quadrants; the sim's group check is partition-blind so
                    # it must be skipped (values verified exact in CoreSim).
                    nc.tensor.matmul(pooled0, et, h4[:, f, :512],
                                     start=st, stop=sp, skip_group_check=True)
                    nc.tensor.matmul(pooled1, et, h4[:, f, 512:],
                                     start=st, stop=sp, skip_group_check=True)
                    nc.tensor.matmul(denom, et, ones8,
                                     start=st, stop=sp, skip_group_check=True)
                del st_[c]

            sched = ((s4c, 5), (s0, -1), (s1, 0), (s2, 1), (s3, 2),
                     (s4, 3), (s4b, 4), (s5, 5))
            for i in range(-1, nch + 5):
                for stage, off in sched:
                    c = i - off
                    if 0 <= c < nch:
                        stage(c)

            # ---- final per-utterance MLP ----
            W6s = load_w(W6_d, "W6b", BF16)
            b6s = wpool.tile([1, HID], BF16, tag="b6")
            nc.sync.dma_start(b6s[:], b6_d.ap())

            # 1/denom: copy the [64:96] psum quadrant to SBUF, DMA-shift it
            # to partitions 0:32 and 32:64 (engines can't move across lanes)
            dtmp = fpool.tile([3 * S, 1], F32, tag="dtmp")
            nc.vector.tensor_copy(out=dtmp[2 * S:3 * S, 0:1],
                                  in_=denom[:, 0:1])
            fc = colpool.tile([2 * S, 4], F32, tag="col")
            nc.sync.dma_start(fc[0:S, 0:1], dtmp[2 * S:3 * S, 0:1])
            nc.sync.dma_start(fc[S:2 * S, 0:1], dtmp[2 * S:3 * S, 0:1])
            nc.vector.reciprocal(fc[:, 1:2], fc[:, 0:1])

            # pooled (normalized) in f32 for the PE transpose; pooled1 is
            # scaled in place at partitions 32:64, then DMA-shifted down
            pooled_sb = fpool.tile([S, HID], F32, tag="pooled")
            pstg = fpool.tile([2 * S, 512], F32, tag="pstg")
            nc.vector.tensor_scalar_mul(pooled_sb[:, :512], pooled0,
                                        fc[0:S, 1:2])
            nc.vector.tensor_scalar_mul(pstg[S:2 * S, :], pooled1,
                                        fc[S:2 * S, 1:2])
            nc.sync.dma_start(pooled_sb[:, 512:], pstg[S:2 * S, :])

            # transpose pooled -> pooledT [hid, seg] (bf16 via cast copies)
            identf = fpool.tile([S, S], F32, tag="identf")
            nc.vector.tensor_copy(out=identf[:], in_=ident)
            tposed = fpool.tile([P, KS, 2 * S], BF16, tag="tposed")
            pooledT = tposed[:, :, :S]
            gT = tposed[:, :, S:]
            for k in range(KS):
                pst = psL.tile([P, CH], F32, tag="mm")
                nc.tensor.transpose(pst[:, :S], pooled_sb[:, k * P:(k + 1) * P],
                                    identf[:])
                nc.vector.tensor_copy(out=pooledT[:, k, :], in_=pst[:, :S])

            # g = relu(pooled @ W6 + b6)   (seg-major [S, HID], bf16)
            g_sb = fpool.tile([S, HID], BF16, tag="g")
            for n in range(2):
                psg = psL.tile([P, CH], F32, tag="mm")
                for k in range(KS):
                    nc.tensor.matmul(psg[:S, :], pooledT[:, k, :],
                                     W6s[:, k, n * 512:(n + 1) * 512],
                                     start=(k == 0), stop=False)
                nc.tensor.matmul(psg[:S, :], ones_row,
                                 b6s[:, n * 512:(n + 1) * 512],
                                 start=False, stop=True)
                nc.scalar.activation(g_sb[:, n * 512:(n + 1) * 512],
                                     psg[:S, :], RELU)

            # gT [hid, seg] (transpose back via f32 staging)
            gf = fpool.tile([S, HID], F32, tag="gf")
            nc.vector.tensor_copy(out=gf[:], in_=g_sb[:])
            for k in range(KS):
                pst = psL.tile([P, CH], F32, tag="mm")
                nc.tensor.transpose(pst[:, :S], gf[:, k * P:(k + 1) * P],
                                    identf[:])
                nc.vector.tensor_copy(out=gT[:, k, :], in_=pst[:, :S])

            # out = g @ W7 + b7
            pso = psL.tile([P, CH], F32, tag="mm")
            for k in range(KS):
                nc.tensor.matmul(pso[:S, :NCLS], gT[:, k, :], W7v[:, k, :],
                                 start=(k == 0), stop=False)
            nc.tensor.matmul(pso[:S, :NCLS], ones_row, b7row,
                             start=False, stop=True)
            oc = colpool.tile([S, 16], F32, tag="oc")
            nc.vector.tensor_copy(out=oc[:, :NCLS], in_=pso[:S, :NCLS])
            nc.sync.dma_start(out_d.ap()[:], oc[:, :NCLS])

    nc.compile()
    return nc


def prepare_inputs(x, W1, b1, W2, b2, W3, b3, W4, b4, W5, b5, W6, b6, W7, b7,
                   lengths):
    """Host-side sharding/packing. Returns (in_maps, bins, m_pad)."""
    x = np.ascontiguousarray(np.asarray(x, dtype=np.float32))
    lengths = np.asarray(lengths)
    total = x.shape[0]
    seg_ids = _segment_ids(lengths, total)
    counts = np.bincount(seg_ids, minlength=NSEG).astype(np.int64)
    starts = np.zeros(NSEG + 1, dtype=np.int64)
    starts[1:] = np.cumsum(counts)

    bins = _balance_segments(counts)
    core_frames = [int(sum(counts[s] for s in b)) for b in bins]
    m_pad = ((max(core_frames) + CH - 1) // CH) * CH
    frt = m_pad // P

    W1p = np.zeros((P, HID), dtype=np.float32)
    W1p[:FEAT] = np.asarray(W1, dtype=np.float32)
    W1p[FEAT] = np.asarray(b1, dtype=np.float32)

    def dr_pack(W, dt):
        """[1024, 1024] -> [128, 8, 1024] with Wq[p, k, m] = W[k*128+p, m]."""
        Wf = np.asarray(W, np.float32).reshape(KS, P, HID)
        return np.ascontiguousarray(Wf.transpose(1, 0, 2)).astype(dt)

    misc = np.zeros((P, 32), dtype=np.float32)
    misc[:, MC_B2:MC_B2 + KS] = np.asarray(b2, np.float32).reshape(KS, P).T
    misc[:, MC_B3:MC_B3 + KS] = np.asarray(b3, np.float32).reshape(KS, P).T
    misc[:, MC_B5] = np.float32(np.asarray(b5, np.float32).reshape(-1)[0])

    cbf = np.zeros((P, 128), dtype=np.float32)
    cbf[:, CB_ONES8:CB_ONES8 + 8] = 1.0
    cbf[:SEGS_PER_CORE, CB_ID:CB_ID + SEGS_PER_CORE] = np.eye(
        SEGS_PER_CORE, dtype=np.float32)
    cbf[:, CB_W7:CB_W7 + KS * NCLS] = np.asarray(W7, np.float32).reshape(
        KS, P, NCLS).transpose(1, 0, 2).reshape(P, KS * NCLS)

    rwb = np.zeros((1, 64), dtype=np.float32)
    rwb[0, RW_B7:RW_B7 + NCLS] = np.asarray(b7, np.float32).reshape(-1)
    rwb[0, RW_ONES:RW_ONES + SEGS_PER_CORE] = 1.0

    c8 = np.zeros((1, 2, 1536), dtype=np.float32)
    c8[0, :, 0:P] = 1.0
    b4f = np.asarray(b4, np.float32).reshape(-1)
    b4hi = b4f.astype(E4NP).astype(np.float32)
    b4lo = (b4f - b4hi).astype(E4NP).astype(np.float32)
    c8[0, 0, 512:1536] = b4hi
    c8[0, 1, 512:1536] = b4lo

    shared = dict(
        W1p=W1p.astype(BFNP),
        W2q=dr_pack(W2, E4NP),
        W3q=dr_pack(W3, E4NP),
        W4q=dr_pack(W4, E4NP),
        W5rep=np.broadcast_to(np.asarray(W5, np.float32).reshape(1, HID),
                              (P, HID)).astype(BFNP),
        W6b=dr_pack(W6, BFNP),
        b6r=np.asarray(b6, np.float32).reshape(1, HID).astype(BFNP),
        miscc=misc,
        cbf=cbf.astype(BFNP),
        c8=c8.astype(E4NP),
        rwb=rwb.astype(BFNP),
    )

    in_maps = []
    for core in range(NCORES):
        segs = bins[core]
        xs = [x[starts[s]:starts[s + 1]] for s in segs]
        xcat = np.concatenate(xs, axis=0) if xs else np.zeros((0, FEAT), np.float32)
        n = xcat.shape[0]
        xT = np.zeros((P, m_pad), dtype=np.float32)
        xT[:FEAT, :n] = xcat.T
        xT[FEAT, :n] = 1.0  # constant feature -> b1
        A = np.zeros((m_pad, SEGS_PER_CORE), dtype=np.float32)
        off = 0
        for j, s in enumerate(segs):
            ln = int(counts[s])
            A[off:off + ln, j] = 1.0
            off += ln
        im = dict(shared)
        im["xT"] = xT.astype(BFNP)
        # partition-major layout [P, frt, S]: Ah[p, t, s] = A[t*128 + p, s]
        im["Amat"] = np.ascontiguousarray(
            A.reshape(frt, P, SEGS_PER_CORE).transpose(1, 0, 2)).astype(BFNP)
        in_maps.append(im)
    return in_maps, bins, m_pad


_PROGRAM_CACHE: dict[int, object] = {}


def kernel(**inputs) -> np.ndarray:
    in_maps, bins, m_pad = prepare_inputs(**inputs)
    nc = _PROGRAM_CACHE.get(m_pad)
    if nc is None:
        nc = _build_program(m_pad)
        _PROGRAM_CACHE[m_pad] = nc
    res = run_bass_kernel_spmd(nc, in_maps, core_ids=list(range(NCORES)))
    out = np.zeros((NSEG, NCLS), dtype=np.float32)
    for core in range(NCORES):
        out[bins[core]] = res.results[core]["out"]
    return out


# revision 39
# speedup vs baseline: 1.0413x; 1.0325x over previous
"""Trainium2 Bass kernel for nn_Dnn_with_Attention (ragged attention-pooled DNN).

Contract: kernel(**inputs) takes FULL unsharded numpy inputs (keys as in
reference.setup_inputs()) and returns the FULL [256, 10] float32 output.

Strategy (data-parallel over utterances, 8 NeuronCores):
  - Host: greedily balance the 256 segments over 8 cores (32 whole segments
    each), gather each core's frames, transpose x to feature-major
    bf16 [128(feat-padded), M_PAD] and build a per-frame one-hot segment
    membership matrix A (bf16).  A row of ones is appended as feature 78 so
    b1 folds into W1.
  - Device (per core): L1 in bf16 (feature-major, [1024, frames]); L2/L3/L4
    run in fp8 e4m3 with MatmulPerfMode.DoubleRow (two 128-K slices per
    instruction at 0.5 cycles/row, ~4x the f32r rate).  Weights W2/W3/W4 are
    host-quantized to e4m3; inter-layer activations are written as e4m3
    directly by the relu ops.  L4 produces frame-major h4 in bf16; b4 is
    added via a DoubleRow matmul against a host-packed (hi, lo) e4m3 pair so
    the quantization error cancels.  Scores: GpSimd computes h4 * W5 (SBUF
    only; GPSIMD cannot touch PSUM), DVE reduces the innermost axis, Scalar
    takes the exp; e = max(exp(score + b5), 1) folds the relu.  Segment
    softmax pooling is small PE matmuls E.T @ h4 (E = A * e, bf16)
    accumulated into persistent PSUM across all chunks; the denominator
    comes from E.T @ ones into the same PSUM bank at a different partition
    quadrant.  The final per-utterance MLP runs once at the end in bf16.
  - The whole program is emitted statically as a 6-stage software pipeline
    over chunks (L1 / L2 / L3 / L4+score-product / score-tail / pooling,
    each one iteration apart), so every matmul -> relu -> next-layer
    dependency crosses a full ~19us iteration and the PE never waits on the
    other engines; this also keeps the PE out of its low p-states.  Relu
    drains alternate between Scalar and DVE per psum tile so each layer
    phase drains as fast as the PE fills it.
"""

import sys

sys.path.insert(0, "/opt/trn_rl_repo")

import numpy as np
import ml_dtypes

import concourse.bass as bass
import concourse.mybir as mybir
import concourse.tile as tile
from concourse import bacc
from concourse.bass_utils import run_bass_kernel_spmd

P = 128
FEAT = 78
HID = 1024
NCLS = 10
NSEG = 256
NCORES = 8
SEGS_PER_CORE = NSEG // NCORES
CH = 512           # frames per chunk (free dim of the layer matmuls)
FRT_PER_CH = CH // P
KS = HID // P      # 8 k-subtiles
F32 = mybir.dt.float32
F32R = mybir.dt.float32r
BF16 = mybir.dt.bfloat16
F8 = mybir.dt.float8e4
DR = mybir.MatmulPerfMode.DoubleRow
E4NP = ml_dtypes.float8_e4m3
BFNP = ml_dtypes.bfloat16

# misc constant tile column layout ([128, 32] f32, host-packed)
MC_B2 = 0          # cols 0..7   : b2 striped [128, 8]
MC_B3 = 8          # cols 8..15  : b3 striped
MC_B5 = 17         # col 17      : b5 replicated down partitions
# bf16 const tile ([128, 96])
CB_ONES8 = 0       # cols 0..7  : ones (denom matmul rhs)
CB_ID = 8          # cols 8..39, rows 0..31: 32x32 identity
CB_W7 = 40         # cols 40..119?? keep within 96: W7 as [128, 8, 10] -> 80 cols
# fp8 const row ([1, 2, 1536]): ones pair + b4 (hi, lo) pair
# row layout [1, 2, 1536]: [:, :, 0:128] ones, [:, :, 512:1536] b4 hi/lo
# simpler: two fields side by side, see prepare_inputs
# bf16 row consts ([1, 64])
RW_B7 = 0          # cols 0..9 : b7
RW_ONES = 16       # cols 16..48 : ones row (bias matmuls, final MLP)


def _segment_ids(lengths: np.ndarray, total: int) -> np.ndarray:
    """Replicate jnp.repeat(arange(n), lengths, total_repeat_length=total)."""
    lengths = np.asarray(lengths, dtype=np.int64)
    seg = np.repeat(np.arange(lengths.shape[0], dtype=np.int32), np.maximum(lengths, 0))
    if seg.shape[0] >= total:
        return seg[:total]
    pad_val = seg[-1] if seg.shape[0] > 0 else np.int32(0)
    return np.concatenate([seg, np.full(total - seg.shape[0], pad_val, np.int32)])


def _balance_segments(lengths: np.ndarray) -> list[list[int]]:
    """Assign 256 segments to 8 cores, 32 each, minimizing max frame count."""
    order = np.argsort(-lengths, kind="stable")
    loads = [0] * NCORES
    bins: list[list[int]] = [[] for _ in range(NCORES)]
    for s in order:
        cands = [c for c in range(NCORES) if len(bins[c]) < SEGS_PER_CORE]
        c = min(cands, key=lambda c: (loads[c], c))
        bins[c].append(int(s))
        loads[c] += int(lengths[s])
    for b in bins:
        b.sort()
    return bins


def _build_program(m_pad: int):
    """Emit the Bass/Tile program for one core with m_pad frames (static)."""
    nch = m_pad // CH
    frt = m_pad // P
    S = SEGS_PER_CORE

    nc = bacc.Bacc("TRN2", target_bir_lowering=False, debug=False,
                   num_devices=NCORES)

    xT_d = nc.dram_tensor("xT", [P, m_pad], BF16, kind="ExternalInput")
    A_d = nc.dram_tensor("Amat", [P, frt, S], BF16, kind="ExternalInput")
    W1_d = nc.dram_tensor("W1p", [P, HID], BF16, kind="ExternalInput")
    W2_d = nc.dram_tensor("W2q", [P, KS, HID], F8, kind="ExternalInput")
    W3_d = nc.dram_tensor("W3q", [P, KS, HID], F8, kind="ExternalInput")
    W4_d = nc.dram_tensor("W4q", [P, KS, HID], F8, kind="ExternalInput")
    W5_d = nc.dram_tensor("W5rep", [P, HID], BF16, kind="ExternalInput")
    W6_d = nc.dram_tensor("W6b", [P, KS, HID], BF16, kind="ExternalInput")
    b6_d = nc.dram_tensor("b6r", [1, HID], BF16, kind="ExternalInput")
    misc_d = nc.dram_tensor("miscc", [P, 32], F32, kind="ExternalInput")
    cbf_d = nc.dram_tensor("cbf", [P, 128], BF16, kind="ExternalInput")
    c8_d = nc.dram_tensor("c8", [1, 2, 1536], F8, kind="ExternalInput")
    rw_d = nc.dram_tensor("rwb", [1, 64], BF16, kind="ExternalInput")
    out_d = nc.dram_tensor("out", [S, NCLS], F32, kind="ExternalOutput")

    RELU = mybir.ActivationFunctionType.Relu
    EXP = mybir.ActivationFunctionType.Exp
    MULT = mybir.AluOpType.mult
    ADD = mybir.AluOpType.add
    MAX = mybir.AluOpType.max

    with tile.TileContext(nc) as tc:
        with (
            tc.tile_pool(name="wpool", bufs=1) as wpool,
            tc.tile_pool(name="xpool", bufs=6) as xpool,
            tc.tile_pool(name="apool", bufs=10) as apool,
            tc.tile_pool(name="h1pool", bufs=2) as h1pool,
            tc.tile_pool(name="h2pool", bufs=2) as h2pool,
            tc.tile_pool(name="h3pool", bufs=2) as h3pool,
            tc.tile_pool(name="h4pool", bufs=4) as h4pool,
            tc.tile_pool(name="scrpool", bufs=4) as scrpool,
            tc.tile_pool(name="colpool", bufs=4) as colpool,
            tc.tile_pool(name="epool", bufs=4) as epool,
            tc.tile_pool(name="fpool", bufs=1) as fpool,
            tc.tile_pool(name="psL", bufs=7, space="PSUM") as psL,
            tc.tile_pool(name="psAcc", bufs=1, space="PSUM") as psAcc,
        ):
            # ---- resident constants/weights ----
            W1s = wpool.tile([P, HID], BF16, tag="W1")
            nc.sync.dma_start(W1s[:], W1_d.ap())

            def load_w(d, tagp, dt):
                t = wpool.tile([P, KS, HID], dt, tag=tagp)
                for k in range(KS):
                    nc.sync.dma_start(t[:, k, :], d.ap()[:, k, :])
                return t

            misc = wpool.tile([P, 32], F32, tag="misc")
            nc.sync.dma_start(misc[:], misc_d.ap())
            cbf = wpool.tile([P, 128], BF16, tag="cbf")
            nc.sync.dma_start(cbf[:], cbf_d.ap())
            c8 = wpool.tile([1, 2, 1536], F8, tag="c8")
            nc.sync.dma_start(c8[:], c8_d.ap())
            rwb = wpool.tile([1, 64], BF16, tag="rwb")
            nc.sync.dma_start(rwb[:], rw_d.ap())

            # prefetch the first six chunks' x/A BEFORE the big weight
            # loads: L1 of chunk 0 only needs W1 + x(0), so the PE can
            # start ~2us in instead of waiting ~21us for all weights.
            pre_ = {}
            for c0 in range(min(6, nch)):
                xt0 = xpool.tile([P, CH], BF16, tag="x")
                nc.sync.dma_start(xt0[:], xT_d.ap()[:, c0 * CH:(c0 + 1) * CH])
                ag0 = apool.tile([P, FRT_PER_CH, S], BF16, tag="A")
                nc.sync.dma_start(
                    ag0[:], A_d.ap()[:, c0 * FRT_PER_CH:
                                     (c0 + 1) * FRT_PER_CH, :])
                pre_[c0] = {"ag": ag0, "xt": xt0}

            W2s = load_w(W2_d, "W2q", F8)
            W3s = load_w(W3_d, "W3q", F8)
            W4s = load_w(W4_d, "W4q", F8)
            W5s4 = wpool.tile([P, FRT_PER_CH, HID], BF16, tag="W5")
            for f in range(FRT_PER_CH):
                nc.sync.dma_start(W5s4[:, f, :], W5_d.ap())

            b5col = misc[:, MC_B5:MC_B5 + 1]
            ones8 = cbf[:, CB_ONES8:CB_ONES8 + 8]
            ident = cbf[:S, CB_ID:CB_ID + S]
            W7v = cbf[:, CB_W7:CB_W7 + KS * NCLS].rearrange(
                "p (o c) -> p o c", c=NCLS)
            b7row = rwb[:, RW_B7:RW_B7 + NCLS]
            ones_row = rwb[:, RW_ONES:RW_ONES + S]
            ones_pair8 = c8[:, :, 0:P]          # [1, 2, 128] of ones (fp8)
            b4pair = c8[:, :, 512:1536]         # [1, 2, 1024] b4 (hi, lo)

            # persistent PSUM accumulators, all in ONE bank at different
            # partition quadrants (frees a bank for deeper psum rotation):
            #   pooled0 [0:32, 0:512], pooled1 [32:64, 0:512],
            #   denom [64:96, 0:8]
            acc0 = psAcc.tile([P, 512], F32, tag="acc0")
            pooled0 = acc0[0:S, :]
            pooled1 = acc0[S:2 * S, :]
            denom = acc0[2 * S:3 * S, 0:8]

            # ---- main pass: 5-stage software pipeline over chunks ----
            # Stage k of chunk c runs in iteration c+k, so every cross-engine
            # dependency (matmul -> relu -> next layer's matmul) has a full
            # iteration (~15us) of slack and the PE never waits on the relus.
            st_ = {}   # per-chunk tile state

            def relu_ps(out, in_, bias, eng):
                """relu(in_ + bias) -> out (fp8/bf16 cast) on Scalar or DVE."""
                if eng == 0:
                    nc.scalar.activation(out, in_, RELU,
                                         bias=0.0 if bias is None else bias)
                elif bias is None:
                    nc.vector.tensor_scalar_max(out, in_, 0.0)
                else:
                    nc.vector.tensor_scalar(out=out, in0=in_, scalar1=bias,
                                            scalar2=0.0, op0=ADD, op1=MAX)

            def s0(c):  # prefetch x/A one iteration ahead of the L1 use
                if c in pre_:
                    st_[c] = pre_[c]
                    return
                xt = xpool.tile([P, CH], BF16, tag="x")
                nc.sync.dma_start(xt[:], xT_d.ap()[:, c * CH:(c + 1) * CH])
                ag = apool.tile([P, FRT_PER_CH, S], BF16, tag="A")
                nc.sync.dma_start(
                    ag[:], A_d.ap()[:, c * FRT_PER_CH:(c + 1) * FRT_PER_CH, :])
                st_[c] = {"ag": ag, "xt": xt}

            def s1(c):  # L1 (bf16) -> h1 fp8 (per-m relu, alternating eng)
                xt = st_[c]["xt"]
                h1 = h1pool.tile([P, KS, CH], F8, tag="h1")
                for m in range(KS):
                    ps = psL.tile([P, CH], F32, tag="mm")
                    nc.tensor.matmul(ps[:], W1s[:, m * P:(m + 1) * P], xt[:],
                                     start=True, stop=True)
                    relu_ps(h1[:, m, :], ps[:], None,
                            0 if m in (0, 2, 4, 6, 7) else 1)
                st_[c]["h1"] = h1

            def _mid_layer(c, Ws, hin_key, hout_key, pool, boff, flip):
                """L2/L3: fp8 DoubleRow + per-m relu(+bias), alternating
                engines per psum-tile half so the drain keeps up with PE."""
                hin = st_[c][hin_key]
                hout = pool.tile([P, KS, CH], F8, tag=hout_key)
                for m in range(KS):
                    ps = psL.tile([P, CH], F32, tag="mm")
                    for t in range(KS // 2):
                        nc.tensor.matmul(
                            ps[:],
                            Ws[:, 2 * t:2 * t + 2, m * P:(m + 1) * P],
                            hin[:, 2 * t:2 * t + 2, :],
                            start=(t == 0), stop=(t == KS // 2 - 1),
                            perf_mode=DR)
                    relu_ps(hout[:, m, :], ps[:],
                            misc[:, boff + m:boff + m + 1], (m + flip) % 2)
                st_[c][hout_key] = hout

            def s2(c):
                _mid_layer(c, W2s, "h1", "h2", h2pool, MC_B2, 0)

            def s3(c):
                _mid_layer(c, W3s, "h2", "h3", h3pool, MC_B3, 1)

            def s4(c):  # L4 fp8 DoubleRow -> h4 bf16; scores -> E (bf16)
                h3 = st_[c]["h3"]
                ag = st_[c]["ag"]
                h4 = h4pool.tile([P, FRT_PER_CH, HID], BF16, tag="h4")
                for f in range(FRT_PER_CH):
                    for n in range(2):
                        ps4 = psL.tile([P, CH], F32, tag="mm")
                        nc.tensor.matmul(ps4[:], ones_pair8,
                                         b4pair[:, :, n * 512:(n + 1) * 512],
                                         start=True, stop=False, perf_mode=DR)
                        for t in range(KS // 2):
                            nc.tensor.matmul(
                                ps4[:],
                                h3[:, 2 * t:2 * t + 2, f * P:(f + 1) * P],
                                W4s[:, 2 * t:2 * t + 2, n * 512:(n + 1) * 512],
                                start=False, stop=(t == KS // 2 - 1),
                                perf_mode=DR)
                        relu_ps(h4[:, f, n * 512:(n + 1) * 512], ps4[:], None,
                                0 if (2 * f + n) in (0, 2, 4, 6, 7) else 1)
                # scores, fully batched: one GpSimd product over all four
                # f-tiles, one DVE reduce (innermost axis), one exp, one
                # clamp; per-f E columns on GpSimd.  E is consumed by the
                # pooling matmuls one iteration later.
                scr = scrpool.tile([P, FRT_PER_CH, HID], BF16, tag="scr")
                for f in range(FRT_PER_CH):
                    # drain tail (last two chunks): the product runs on DVE
                    # in the fast bf16 2x mode -- once the main loop ends
                    # nothing hides the serial GpSimd product chain
                    nc.vector.tensor_mul(scr[:, f, :], h4[:, f, :],
                                         W5s4[:, f, :])
                st_[c]["h4"] = h4
                st_[c]["scr"] = scr

            def s4b(c):  # score reduces at the very END of the DVE stream
                # (their products are an iteration old, so they never
                # head-of-line-block the DVE act drains).
                scr = st_[c]["scr"]
                ct = colpool.tile([P, FRT_PER_CH], F32, tag="ctb")
                for f in range(FRT_PER_CH):
                    nc.vector.tensor_reduce(out=ct[:, f:f + 1],
                                            in_=scr[:, f, :],
                                            axis=mybir.AxisListType.X, op=ADD)
                st_[c]["ct"] = ct

            def s4c(c):  # exp/clamp/E at the START of the next iteration:
                # exp leads the Scalar queue (its reduces are an iteration
                # old) so Pool's clamp/E never wait deep into the iteration
                # and Pool's next product batch starts on time.
                ag = st_[c]["ag"]
                ct = st_[c]["ct"]
                etg = epool.tile([P, FRT_PER_CH, S], BF16, tag="E")
                ec = colpool.tile([P, 2 * FRT_PER_CH], F32, tag="ec")
                nc.scalar.activation(ec[:, :FRT_PER_CH], ct[:], EXP,
                                     bias=b5col)
                nc.gpsimd.tensor_scalar_max(ec[:, FRT_PER_CH:],
                                            ec[:, :FRT_PER_CH], 1.0)
                for f in range(FRT_PER_CH):
                    nc.gpsimd.tensor_scalar_mul(
                        etg[:, f, :], ag[:, f, :],
                        ec[:, FRT_PER_CH + f:FRT_PER_CH + f + 1])
                st_[c]["et"] = etg

            def s5(c):  # pooling matmuls (persistent PSUM accumulation)
                h4 = st_[c]["h4"]
                etg = st_[c]["et"]
                first = c == 0
                last = c == nch - 1
                for f in range(FRT_PER_CH):
                    et = etg[:, f, :]
                    st = bool(first and f == 0)
                    sp = bool(last and f == FRT_PER_CH - 1)
                    # pooled0/denom share a PSUM bank at different partition
                    # quadrants; the sim's group check is partition-blind so
                    # it must be skipped (values verified exact in CoreSim).
                    nc.tensor.matmul(pooled0, et, h4[:, f, :512],
                                     start=st, stop=sp, skip_group_check=True)
                    nc.tensor.matmul(pooled1, et, h4[:, f, 512:],
                                     start=st, stop=sp, skip_group_check=True)
                    nc.tensor.matmul(denom, et, ones8,
                                     start=st, stop=sp, skip_group_check=True)
                del st_[c]

            sched = ((s4c, 5), (s0, -1), (s1, 0), (s2, 1), (s3, 2),
                     (s4, 3), (s4b, 4), (s5, 5))
            for i in range(-1, nch + 5):
                for stage, off in sched:
                    c = i - off
                    if 0 <= c < nch:
                        stage(c)

            # ---- final per-utterance MLP ----
            W6s = load_w(W6_d, "W6b", BF16)
            b6s = wpool.tile([1, HID], BF16, tag="b6")
            nc.sync.dma_start(b6s[:], b6_d.ap())

            # 1/denom: copy the [64:96] psum quadrant to SBUF, DMA-shift it
            # to partitions 0:32 and 32:64 (engines can't move across lanes)
            dtmp = fpool.tile([3 * S, 1], F32, tag="dtmp")
            nc.vector.tensor_copy(out=dtmp[2 * S:3 * S, 0:1],
                                  in_=denom[:, 0:1])
            fc = colpool.tile([2 * S, 4], F32, tag="col")
            nc.sync.dma_start(fc[0:S, 0:1], dtmp[2 * S:3 * S, 0:1])
            nc.sync.dma_start(fc[S:2 * S, 0:1], dtmp[2 * S:3 * S, 0:1])
            nc.vector.reciprocal(fc[:, 1:2], fc[:, 0:1])

            # pooled (normalized) in f32 for the PE transpose; pooled1 is
            # scaled in place at partitions 32:64, then DMA-shifted down
            pooled_sb = fpool.tile([S, HID], F32, tag="pooled")
            pstg = fpool.tile([2 * S, 512], F32, tag="pstg")
            nc.vector.tensor_scalar_mul(pooled_sb[:, :512], pooled0,
                                        fc[0:S, 1:2])
            nc.vector.tensor_scalar_mul(pstg[S:2 * S, :], pooled1,
                                        fc[S:2 * S, 1:2])
            nc.sync.dma_start(pooled_sb[:, 512:], pstg[S:2 * S, :])

            # transpose pooled -> pooledT [hid, seg] (bf16 via cast copies)
            identf = fpool.tile([S, S], F32, tag="identf")
            nc.vector.tensor_copy(out=identf[:], in_=ident)
            tposed = fpool.tile([P, KS, 2 * S], BF16, tag="tposed")
            pooledT = tposed[:, :, :S]
            gT = tposed[:, :, S:]
            for k in range(KS):
                pst = psL.tile([P, CH], F32, tag="mm")
                nc.tensor.transpose(pst[:, :S], pooled_sb[:, k * P:(k + 1) * P],
                                    identf[:])
                nc.vector.tensor_copy(out=pooledT[:, k, :], in_=pst[:, :S])

            # g = relu(pooled @ W6 + b6)   (seg-major [S, HID], bf16)
            g_sb = fpool.tile([S, HID], BF16, tag="g")
            for n in range(2):
                psg = psL.tile([P, CH], F32, tag="mm")
                for k in range(KS):
                    nc.tensor.matmul(psg[:S, :], pooledT[:, k, :],
                                     W6s[:, k, n * 512:(n + 1) * 512],
                                     start=(k == 0), stop=False)
                nc.tensor.matmul(psg[:S, :], ones_row,
                                 b6s[:, n * 512:(n + 1) * 512],
                                 start=False, stop=True)
                nc.scalar.activation(g_sb[:, n * 512:(n + 1) * 512],
                                     psg[:S, :], RELU)

            # gT [hid, seg] (transpose back via f32 staging)
            gf = fpool.tile([S, HID], F32, tag="gf")
            nc.vector.tensor_copy(out=gf[:], in_=g_sb[:])
            for k in range(KS):
                pst = psL.tile([P, CH], F32, tag="mm")
                nc.tensor.transpose(pst[:, :S], gf[:, k * P:(k + 1) * P],
                                    identf[:])
                nc.vector.tensor_copy(out=gT[:, k, :], in_=pst[:, :S])

            # out = g @ W7 + b7
            pso = psL.tile([P, CH], F32, tag="mm")
            for k in range(KS):
                nc.tensor.matmul(pso[:S, :NCLS], gT[:, k, :], W7v[:, k, :],
                                 start=(k == 0), stop=False)
            nc.tensor.matmul(pso[:S, :NCLS], ones_row, b7row,
                             start=False, stop=True)
            oc = colpool.tile([S, 16], F32, tag="oc")
            nc.vector.tensor_copy(out=oc[:, :NCLS], in_=pso[:S, :NCLS])
            nc.sync.dma_start(out_d.ap()[:], oc[:, :NCLS])

    nc.compile()
    return nc


def prepare_inputs(x, W1, b1, W2, b2, W3, b3, W4, b4, W5, b5, W6, b6, W7, b7,
                   lengths):
    """Host-side sharding/packing. Returns (in_maps, bins, m_pad)."""
    x = np.ascontiguousarray(np.asarray(x, dtype=np.float32))
    lengths = np.asarray(lengths)
    total = x.shape[0]
    seg_ids = _segment_ids(lengths, total)
    counts = np.bincount(seg_ids, minlength=NSEG).astype(np.int64)
    starts = np.zeros(NSEG + 1, dtype=np.int64)
    starts[1:] = np.cumsum(counts)

    bins = _balance_segments(counts)
    core_frames = [int(sum(counts[s] for s in b)) for b in bins]
    m_pad = ((max(core_frames) + CH - 1) // CH) * CH
    frt = m_pad // P

    W1p = np.zeros((P, HID), dtype=np.float32)
    W1p[:FEAT] = np.asarray(W1, dtype=np.float32)
    W1p[FEAT] = np.asarray(b1, dtype=np.float32)

    def dr_pack(W, dt):
        """[1024, 1024] -> [128, 8, 1024] with Wq[p, k, m] = W[k*128+p, m]."""
        Wf = np.asarray(W, np.float32).reshape(KS, P, HID)
        return np.ascontiguousarray(Wf.transpose(1, 0, 2)).astype(dt)

    misc = np.zeros((P, 32), dtype=np.float32)
    misc[:, MC_B2:MC_B2 + KS] = np.asarray(b2, np.float32).reshape(KS, P).T
    misc[:, MC_B3:MC_B3 + KS] = np.asarray(b3, np.float32).reshape(KS, P).T
    misc[:, MC_B5] = np.float32(np.asarray(b5, np.float32).reshape(-1)[0])

    cbf = np.zeros((P, 128), dtype=np.float32)
    cbf[:, CB_ONES8:CB_ONES8 + 8] = 1.0
    cbf[:SEGS_PER_CORE, CB_ID:CB_ID + SEGS_PER_CORE] = np.eye(
        SEGS_PER_CORE, dtype=np.float32)
    cbf[:, CB_W7:CB_W7 + KS * NCLS] = np.asarray(W7, np.float32).reshape(
        KS, P, NCLS).transpose(1, 0, 2).reshape(P, KS * NCLS)

    rwb = np.zeros((1, 64), dtype=np.float32)
    rwb[0, RW_B7:RW_B7 + NCLS] = np.asarray(b7, np.float32).reshape(-1)
    rwb[0, RW_ONES:RW_ONES + SEGS_PER_CORE] = 1.0

    c8 = np.zeros((1, 2, 1536), dtype=np.float32)
    c8[0, :, 0:P] = 1.0
    b4f = np.asarray(b4, np.float32).reshape(-1)
    b4hi = b4f.astype(E4NP).astype(np.float32)
    b4lo = (b4f - b4hi).astype(E4NP).astype(np.float32)
    c8[0, 0, 512:1536] = b4hi
    c8[0, 1, 512:1536] = b4lo

    shared = dict(
        W1p=W1p.astype(BFNP),
        W2q=dr_pack(W2, E4NP),
        W3q=dr_pack(W3, E4NP),
        W4q=dr_pack(W4, E4NP),
        W5rep=np.broadcast_to(np.asarray(W5, np.float32).reshape(1, HID),
                              (P, HID)).astype(BFNP),
        W6b=dr_pack(W6, BFNP),
        b6r=np.asarray(b6, np.float32).reshape(1, HID).astype(BFNP),
        miscc=misc,
        cbf=cbf.astype(BFNP),
        c8=c8.astype(E4NP),
        rwb=rwb.astype(BFNP),
    )

    in_maps = []
    for core in range(NCORES):
        segs = bins[core]
        xs = [x[starts[s]:starts[s + 1]] for s in segs]
        xcat = np.concatenate(xs, axis=0) if xs else np.zeros((0, FEAT), np.float32)
        n = xcat.shape[0]
        xT = np.zeros((P, m_pad), dtype=np.float32)
        xT[:FEAT, :n] = xcat.T
        xT[FEAT, :n] = 1.0  # constant feature -> b1
        A = np.zeros((m_pad, SEGS_PER_CORE), dtype=np.float32)
        off = 0
        for j, s in enumerate(segs):
            ln = int(counts[s])
            A[off:off + ln, j] = 1.0
            off += ln
        im = dict(shared)
        im["xT"] = xT.astype(BFNP)
        # partition-major layout [P, frt, S]: Ah[p, t, s] = A[t*128 + p, s]
        im["Amat"] = np.ascontiguousarray(
            A.reshape(frt, P, SEGS_PER_CORE).transpose(1, 0, 2)).astype(BFNP)
        in_maps.append(im)
    return in_maps, bins, m_pad


_PROGRAM_CACHE: dict[int, object] = {}


def kernel(**inputs) -> np.ndarray:
    in_maps, bins, m_pad = prepare_inputs(**inputs)
    nc = _PROGRAM_CACHE.get(m_pad)
    if nc is None:
        nc = _build_program(m_pad)
        _PROGRAM_CACHE[m_pad] = nc
    res = run_bass_kernel_spmd(nc, in_maps, core_ids=list(range(NCORES)))
    out = np.zeros((NSEG, NCLS), dtype=np.float32)
    for core in range(NCORES):
        out[bins[core]] = res.results[core]["out"]
    return out


# revision 40
# speedup vs baseline: 1.1232x; 1.0786x over previous
"""Trainium2 Bass kernel for nn_Dnn_with_Attention (ragged attention-pooled DNN).

Contract: kernel(**inputs) takes FULL unsharded numpy inputs (keys as in
reference.setup_inputs()) and returns the FULL [256, 10] float32 output.

Strategy (data-parallel over utterances, 8 NeuronCores):
  - Host: greedily balance the 256 segments over 8 cores (32 whole segments
    each), gather each core's frames, transpose x to feature-major
    bf16 [128(feat-padded), M_PAD] and build a per-frame one-hot segment
    membership matrix A (bf16).  A row of ones is appended as feature 78 so
    b1 folds into W1.
  - Device (per core): L1 in bf16 (feature-major, [1024, frames]); L2/L3/L4
    run in fp8 e4m3 with MatmulPerfMode.DoubleRow (two 128-K slices per
    instruction at 0.5 cycles/row, ~4x the f32r rate).  Weights W2/W3/W4 are
    host-quantized to e4m3; inter-layer activations are written as e4m3
    directly by the relu ops.  L4 produces frame-major h4 in bf16; b4 is
    added via a DoubleRow matmul against a host-packed (hi, lo) e4m3 pair so
    the quantization error cancels.  Scores: GpSimd computes h4 * W5 (SBUF
    only; GPSIMD cannot touch PSUM), DVE reduces the innermost axis, Scalar
    takes the exp; e = max(exp(score + b5), 1) folds the relu.  Segment
    softmax pooling is small PE matmuls E.T @ h4 (E = A * e, bf16)
    accumulated into persistent PSUM across all chunks; the denominator
    comes from E.T @ ones into the same PSUM bank at a different partition
    quadrant.  The final per-utterance MLP runs once at the end in bf16.
  - The whole program is emitted statically as a 6-stage software pipeline
    over chunks (L1 / L2 / L3 / L4+score-product / score-tail / pooling,
    each one iteration apart), so every matmul -> relu -> next-layer
    dependency crosses a full ~19us iteration and the PE never waits on the
    other engines; this also keeps the PE out of its low p-states.  Relu
    drains alternate between Scalar and DVE per psum tile so each layer
    phase drains as fast as the PE fills it.
"""

import sys

sys.path.insert(0, "/opt/trn_rl_repo")

import numpy as np
import ml_dtypes

import concourse.bass as bass
import concourse.mybir as mybir
import concourse.tile as tile
from concourse import bacc
from concourse.bass_utils import run_bass_kernel_spmd

P = 128
FEAT = 78
HID = 1024
NCLS = 10
NSEG = 256
NCORES = 8
SEGS_PER_CORE = NSEG // NCORES
CH = 512           # frames per chunk (free dim of the layer matmuls)
FRT_PER_CH = CH // P
KS = HID // P      # 8 k-subtiles
F32 = mybir.dt.float32
F32R = mybir.dt.float32r
BF16 = mybir.dt.bfloat16
F8 = mybir.dt.float8e4
DR = mybir.MatmulPerfMode.DoubleRow
E4NP = ml_dtypes.float8_e4m3
BFNP = ml_dtypes.bfloat16

# misc constant tile column layout ([128, 32] f32, host-packed)
MC_B2 = 0          # cols 0..7   : b2 striped [128, 8]
MC_B3 = 8          # cols 8..15  : b3 striped
MC_B5 = 17         # col 17      : b5 replicated down partitions
# bf16 const tile ([128, 96])
CB_ONES8 = 0       # cols 0..7  : ones (denom matmul rhs)
CB_ID = 8          # cols 8..39, rows 0..31: 32x32 identity
CB_W7 = 40         # cols 40..119?? keep within 96: W7 as [128, 8, 10] -> 80 cols
# fp8 const row ([1, 2, 1536]): ones pair + b4 (hi, lo) pair
# row layout [1, 2, 1536]: [:, :, 0:128] ones, [:, :, 512:1536] b4 hi/lo
# simpler: two fields side by side, see prepare_inputs
# bf16 row consts ([1, 64])
RW_B7 = 0          # cols 0..9 : b7
RW_ONES = 16       # cols 16..48 : ones row (bias matmuls, final MLP)


def _segment_ids(lengths: np.ndarray, total: int) -> np.ndarray:
    """Replicate jnp.repeat(arange(n), lengths, total_repeat_length=total)."""
    lengths = np.asarray(lengths, dtype=np.int64)
    seg = np.repeat(np.arange(lengths.shape[0], dtype=np.int32), np.maximum(lengths, 0))
    if seg.shape[0] >= total:
        return seg[:total]
    pad_val = seg[-1] if seg.shape[0] > 0 else np.int32(0)
    return np.concatenate([seg, np.full(total - seg.shape[0], pad_val, np.int32)])


def _balance_segments(lengths: np.ndarray) -> list[list[int]]:
    """Assign 256 segments to 8 cores, 32 each, minimizing max frame count."""
    order = np.argsort(-lengths, kind="stable")
    loads = [0] * NCORES
    bins: list[list[int]] = [[] for _ in range(NCORES)]
    for s in order:
        cands = [c for c in range(NCORES) if len(bins[c]) < SEGS_PER_CORE]
        c = min(cands, key=lambda c: (loads[c], c))
        bins[c].append(int(s))
        loads[c] += int(lengths[s])
    for b in bins:
        b.sort()
    return bins


def _build_program(m_pad: int):
    """Emit the Bass/Tile program for one core with m_pad frames (static)."""
    nch = m_pad // CH
    frt = m_pad // P
    S = SEGS_PER_CORE

    nc = bacc.Bacc("TRN2", target_bir_lowering=False, debug=False,
                   num_devices=NCORES)

    xT_d = nc.dram_tensor("xT", [P, m_pad], BF16, kind="ExternalInput")
    A_d = nc.dram_tensor("Amat", [P, frt, S], BF16, kind="ExternalInput")
    W1_d = nc.dram_tensor("W1p", [P, HID], BF16, kind="ExternalInput")
    W2_d = nc.dram_tensor("W2q", [P, KS, HID], F8, kind="ExternalInput")
    W3_d = nc.dram_tensor("W3q", [P, KS, HID], F8, kind="ExternalInput")
    W4_d = nc.dram_tensor("W4q", [P, KS, HID], F8, kind="ExternalInput")
    W5_d = nc.dram_tensor("W5rep", [P, HID], BF16, kind="ExternalInput")
    W6_d = nc.dram_tensor("W6b", [P, KS, HID], BF16, kind="ExternalInput")
    b6_d = nc.dram_tensor("b6r", [1, HID], BF16, kind="ExternalInput")
    misc_d = nc.dram_tensor("miscc", [P, 32], F32, kind="ExternalInput")
    cbf_d = nc.dram_tensor("cbf", [P, 128], BF16, kind="ExternalInput")
    c8_d = nc.dram_tensor("c8", [1, 2, 1536], F8, kind="ExternalInput")
    rw_d = nc.dram_tensor("rwb", [1, 64], BF16, kind="ExternalInput")
    out_d = nc.dram_tensor("out", [S, NCLS], F32, kind="ExternalOutput")

    RELU = mybir.ActivationFunctionType.Relu
    EXP = mybir.ActivationFunctionType.Exp
    MULT = mybir.AluOpType.mult
    ADD = mybir.AluOpType.add
    MAX = mybir.AluOpType.max

    with tile.TileContext(nc) as tc:
        with (
            tc.tile_pool(name="wpool", bufs=1) as wpool,
            tc.tile_pool(name="xpool", bufs=6) as xpool,
            tc.tile_pool(name="apool", bufs=10) as apool,
            tc.tile_pool(name="h1pool", bufs=2) as h1pool,
            tc.tile_pool(name="h2pool", bufs=2) as h2pool,
            tc.tile_pool(name="h3pool", bufs=2) as h3pool,
            tc.tile_pool(name="h4pool", bufs=4) as h4pool,
            tc.tile_pool(name="scrpool", bufs=4) as scrpool,
            tc.tile_pool(name="colpool", bufs=4) as colpool,
            tc.tile_pool(name="epool", bufs=4) as epool,
            tc.tile_pool(name="fpool", bufs=1) as fpool,
            tc.tile_pool(name="psL", bufs=7, space="PSUM") as psL,
            tc.tile_pool(name="psAcc", bufs=1, space="PSUM") as psAcc,
        ):
            # ---- resident constants/weights ----
            W1s = wpool.tile([P, HID], BF16, tag="W1")
            nc.sync.dma_start(W1s[:], W1_d.ap())

            def load_w(d, tagp, dt):
                t = wpool.tile([P, KS, HID], dt, tag=tagp)
                for k in range(KS):
                    nc.sync.dma_start(t[:, k, :], d.ap()[:, k, :])
                return t

            misc = wpool.tile([P, 32], F32, tag="misc")
            nc.sync.dma_start(misc[:], misc_d.ap())
            cbf = wpool.tile([P, 128], BF16, tag="cbf")
            nc.sync.dma_start(cbf[:], cbf_d.ap())
            c8 = wpool.tile([1, 2, 1536], F8, tag="c8")
            nc.sync.dma_start(c8[:], c8_d.ap())
            rwb = wpool.tile([1, 64], BF16, tag="rwb")
            nc.sync.dma_start(rwb[:], rw_d.ap())

            # prefetch the first six chunks' x/A BEFORE the big weight
            # loads: L1 of chunk 0 only needs W1 + x(0), so the PE can
            # start ~2us in instead of waiting ~21us for all weights.
            pre_ = {}
            for c0 in range(min(6, nch)):
                xt0 = xpool.tile([P, CH], BF16, tag="x")
                nc.sync.dma_start(xt0[:], xT_d.ap()[:, c0 * CH:(c0 + 1) * CH])
                ag0 = apool.tile([P, FRT_PER_CH, S], BF16, tag="A")
                nc.sync.dma_start(
                    ag0[:], A_d.ap()[:, c0 * FRT_PER_CH:
                                     (c0 + 1) * FRT_PER_CH, :])
                pre_[c0] = {"ag": ag0, "xt": xt0}

            W2s = load_w(W2_d, "W2q", F8)
            W3s = load_w(W3_d, "W3q", F8)
            W4s = load_w(W4_d, "W4q", F8)
            W5s4 = wpool.tile([P, FRT_PER_CH, HID], BF16, tag="W5")
            for f in range(FRT_PER_CH):
                nc.sync.dma_start(W5s4[:, f, :], W5_d.ap())

            b5col = misc[:, MC_B5:MC_B5 + 1]
            ones8 = cbf[:, CB_ONES8:CB_ONES8 + 8]
            ident = cbf[:S, CB_ID:CB_ID + S]
            W7v = cbf[:, CB_W7:CB_W7 + KS * NCLS].rearrange(
                "p (o c) -> p o c", c=NCLS)
            b7row = rwb[:, RW_B7:RW_B7 + NCLS]
            ones_row = rwb[:, RW_ONES:RW_ONES + S]
            ones_pair8 = c8[:, :, 0:P]          # [1, 2, 128] of ones (fp8)
            b4pair = c8[:, :, 512:1536]         # [1, 2, 1024] b4 (hi, lo)

            # persistent PSUM accumulators, all in ONE bank at different
            # partition quadrants (frees a bank for deeper psum rotation):
            #   pooled0 [0:32, 0:512], pooled1 [32:64, 0:512],
            #   denom [64:96, 0:8]
            acc0 = psAcc.tile([P, 512], F32, tag="acc0")
            pooled0 = acc0[0:S, :]
            pooled1 = acc0[S:2 * S, :]
            denom = acc0[2 * S:3 * S, 0:8]

            # ---- main pass: 5-stage software pipeline over chunks ----
            # Stage k of chunk c runs in iteration c+k, so every cross-engine
            # dependency (matmul -> relu -> next layer's matmul) has a full
            # iteration (~15us) of slack and the PE never waits on the relus.
            st_ = {}   # per-chunk tile state

            def relu_ps(out, in_, bias, eng):
                """relu(in_ + bias) -> out (fp8/bf16 cast) on Scalar or DVE."""
                if eng == 0:
                    nc.scalar.activation(out, in_, RELU,
                                         bias=0.0 if bias is None else bias)
                elif bias is None:
                    nc.vector.tensor_scalar_max(out, in_, 0.0)
                else:
                    nc.vector.tensor_scalar(out=out, in0=in_, scalar1=bias,
                                            scalar2=0.0, op0=ADD, op1=MAX)

            def s0(c):  # prefetch x/A one iteration ahead of the L1 use
                if c in pre_:
                    st_[c] = pre_[c]
                    return
                xt = xpool.tile([P, CH], BF16, tag="x")
                nc.sync.dma_start(xt[:], xT_d.ap()[:, c * CH:(c + 1) * CH])
                ag = apool.tile([P, FRT_PER_CH, S], BF16, tag="A")
                nc.sync.dma_start(
                    ag[:], A_d.ap()[:, c * FRT_PER_CH:(c + 1) * FRT_PER_CH, :])
                st_[c] = {"ag": ag, "xt": xt}

            def s1(c):  # L1 (bf16) -> h1 fp8 (per-m relu, alternating eng)
                xt = st_[c]["xt"]
                h1 = h1pool.tile([P, KS, CH], F8, tag="h1")
                for m in range(KS):
                    ps = psL.tile([P, CH], F32, tag="mm")
                    nc.tensor.matmul(ps[:], W1s[:, m * P:(m + 1) * P], xt[:],
                                     start=True, stop=True)
                    relu_ps(h1[:, m, :], ps[:], None,
                            0 if m in (0, 2, 4, 6, 7) else 1)
                st_[c]["h1"] = h1

            def _mid_layer(c, Ws, hin_key, hout_key, pool, boff, flip):
                """L2/L3: fp8 DoubleRow + per-m relu(+bias), alternating
                engines per psum-tile half so the drain keeps up with PE."""
                hin = st_[c][hin_key]
                hout = pool.tile([P, KS, CH], F8, tag=hout_key)
                for m in range(KS):
                    ps = psL.tile([P, CH], F32, tag="mm")
                    for t in range(KS // 2):
                        nc.tensor.matmul(
                            ps[:],
                            Ws[:, 2 * t:2 * t + 2, m * P:(m + 1) * P],
                            hin[:, 2 * t:2 * t + 2, :],
                            start=(t == 0), stop=(t == KS // 2 - 1),
                            perf_mode=DR)
                    relu_ps(hout[:, m, :], ps[:],
                            misc[:, boff + m:boff + m + 1],
                            1 if (m + flip) % 8 in (1, 4, 6) else 0)
                st_[c][hout_key] = hout

            def s2(c):
                _mid_layer(c, W2s, "h1", "h2", h2pool, MC_B2, 0)

            def s3(c):
                _mid_layer(c, W3s, "h2", "h3", h3pool, MC_B3, 1)

            def s4(c):  # L4 fp8 DoubleRow -> h4 bf16; scores -> E (bf16)
                h3 = st_[c]["h3"]
                ag = st_[c]["ag"]
                h4 = h4pool.tile([P, FRT_PER_CH, HID], BF16, tag="h4")
                for f in range(FRT_PER_CH):
                    for n in range(2):
                        ps4 = psL.tile([P, CH], F32, tag="mm")
                        nc.tensor.matmul(ps4[:], ones_pair8,
                                         b4pair[:, :, n * 512:(n + 1) * 512],
                                         start=True, stop=False, perf_mode=DR)
                        for t in range(KS // 2):
                            nc.tensor.matmul(
                                ps4[:],
                                h3[:, 2 * t:2 * t + 2, f * P:(f + 1) * P],
                                W4s[:, 2 * t:2 * t + 2, n * 512:(n + 1) * 512],
                                start=False, stop=(t == KS // 2 - 1),
                                perf_mode=DR)
                        relu_ps(h4[:, f, n * 512:(n + 1) * 512], ps4[:], None,
                                0 if (2 * f + n) in (0, 2, 4, 6, 7) else 1)
                # scores, fully batched: one GpSimd product over all four
                # f-tiles, one DVE reduce (innermost axis), one exp, one
                # clamp; per-f E columns on GpSimd.  E is consumed by the
                # pooling matmuls one iteration later.
                scr = scrpool.tile([P, FRT_PER_CH, HID], BF16, tag="scr")
                for f in range(FRT_PER_CH):
                    # drain tail (last two chunks): the product runs on DVE
                    # in the fast bf16 2x mode -- once the main loop ends
                    # nothing hides the serial GpSimd product chain
                    nc.vector.tensor_mul(scr[:, f, :], h4[:, f, :],
                                         W5s4[:, f, :])
                st_[c]["h4"] = h4
                st_[c]["scr"] = scr

            def s4b(c):  # score reduces at the very END of the DVE stream
                # (their products are an iteration old, so they never
                # head-of-line-block the DVE act drains).
                scr = st_[c]["scr"]
                ct = colpool.tile([P, FRT_PER_CH], F32, tag="ctb")
                for f in range(FRT_PER_CH):
                    nc.vector.tensor_reduce(out=ct[:, f:f + 1],
                                            in_=scr[:, f, :],
                                            axis=mybir.AxisListType.X, op=ADD)
                st_[c]["ct"] = ct

            def s4c(c):  # exp/clamp/E at the START of the next iteration:
                # exp leads the Scalar queue (its reduces are an iteration
                # old) so Pool's clamp/E never wait deep into the iteration
                # and Pool's next product batch starts on time.
                ag = st_[c]["ag"]
                ct = st_[c]["ct"]
                etg = epool.tile([P, FRT_PER_CH, S], BF16, tag="E")
                ec = colpool.tile([P, 2 * FRT_PER_CH], F32, tag="ec")
                nc.scalar.activation(ec[:, :FRT_PER_CH], ct[:], EXP,
                                     bias=b5col)
                nc.gpsimd.tensor_scalar_max(ec[:, FRT_PER_CH:],
                                            ec[:, :FRT_PER_CH], 1.0)
                for f in range(FRT_PER_CH):
                    nc.gpsimd.tensor_scalar_mul(
                        etg[:, f, :], ag[:, f, :],
                        ec[:, FRT_PER_CH + f:FRT_PER_CH + f + 1])
                st_[c]["et"] = etg

            def s5(c):  # pooling matmuls (persistent PSUM accumulation)
                h4 = st_[c]["h4"]
                etg = st_[c]["et"]
                first = c == 0
                last = c == nch - 1
                for f in range(FRT_PER_CH):
                    et = etg[:, f, :]
                    st = bool(first and f == 0)
                    sp = bool(last and f == FRT_PER_CH - 1)
                    # pooled0/denom share a PSUM bank at different partition
                    # quadrants; the sim's group check is partition-blind so
                    # it must be skipped (values verified exact in CoreSim).
                    nc.tensor.matmul(pooled0, et, h4[:, f, :512],
                                     start=st, stop=sp, skip_group_check=True)
                    nc.tensor.matmul(pooled1, et, h4[:, f, 512:],
                                     start=st, stop=sp, skip_group_check=True)
                    nc.tensor.matmul(denom, et, ones8,
                                     start=st, stop=sp, skip_group_check=True)
                del st_[c]

            sched = ((s4c, 5), (s0, -1), (s1, 0), (s2, 1), (s3, 2),
                     (s4, 3), (s4b, 4), (s5, 5))
            for i in range(-1, nch + 5):
                for stage, off in sched:
                    c = i - off
                    if 0 <= c < nch:
                        stage(c)

            # ---- final per-utterance MLP ----
            W6s = load_w(W6_d, "W6b", BF16)
            b6s = wpool.tile([1, HID], BF16, tag="b6")
            nc.sync.dma_start(b6s[:], b6_d.ap())

            # 1/denom: copy the [64:96] psum quadrant to SBUF, DMA-shift it
            # to partitions 0:32 and 32:64 (engines can't move across lanes)
            dtmp = fpool.tile([3 * S, 1], F32, tag="dtmp")
            nc.vector.tensor_copy(out=dtmp[2 * S:3 * S, 0:1],
                                  in_=denom[:, 0:1])
            fc = colpool.tile([2 * S, 4], F32, tag="col")
            nc.sync.dma_start(fc[0:S, 0:1], dtmp[2 * S:3 * S, 0:1])
            nc.sync.dma_start(fc[S:2 * S, 0:1], dtmp[2 * S:3 * S, 0:1])
            nc.vector.reciprocal(fc[:, 1:2], fc[:, 0:1])

            # pooled (normalized) in f32 for the PE transpose; pooled1 is
            # scaled in place at partitions 32:64, then DMA-shifted down
            pooled_sb = fpool.tile([S, HID], F32, tag="pooled")
            pstg = fpool.tile([2 * S, 512], F32, tag="pstg")
            nc.vector.tensor_scalar_mul(pooled_sb[:, :512], pooled0,
                                        fc[0:S, 1:2])
            nc.vector.tensor_scalar_mul(pstg[S:2 * S, :], pooled1,
                                        fc[S:2 * S, 1:2])
            nc.sync.dma_start(pooled_sb[:, 512:], pstg[S:2 * S, :])

            # transpose pooled -> pooledT [hid, seg] (bf16 via cast copies)
            identf = fpool.tile([S, S], F32, tag="identf")
            nc.vector.tensor_copy(out=identf[:], in_=ident)
            tposed = fpool.tile([P, KS, 2 * S], BF16, tag="tposed")
            pooledT = tposed[:, :, :S]
            gT = tposed[:, :, S:]
            for k in range(KS):
                pst = psL.tile([P, CH], F32, tag="mm")
                nc.tensor.transpose(pst[:, :S], pooled_sb[:, k * P:(k + 1) * P],
                                    identf[:])
                nc.vector.tensor_copy(out=pooledT[:, k, :], in_=pst[:, :S])

            # g = relu(pooled @ W6 + b6)   (seg-major [S, HID], bf16)
            g_sb = fpool.tile([S, HID], BF16, tag="g")
            for n in range(2):
                psg = psL.tile([P, CH], F32, tag="mm")
                for k in range(KS):
                    nc.tensor.matmul(psg[:S, :], pooledT[:, k, :],
                                     W6s[:, k, n * 512:(n + 1) * 512],
                                     start=(k == 0), stop=False)
                nc.tensor.matmul(psg[:S, :], ones_row,
                                 b6s[:, n * 512:(n + 1) * 512],
                                 start=False, stop=True)
                nc.scalar.activation(g_sb[:, n * 512:(n + 1) * 512],
                                     psg[:S, :], RELU)

            # gT [hid, seg] (transpose back via f32 staging)
            gf = fpool.tile([S, HID], F32, tag="gf")
            nc.vector.tensor_copy(out=gf[:], in_=g_sb[:])
            for k in range(KS):
                pst = psL.tile([P, CH], F32, tag="mm")
                nc.tensor.transpose(pst[:, :S], gf[:, k * P:(k + 1) * P],
                                    identf[:])
                nc.vector.tensor_copy(out=gT[:, k, :], in_=pst[:, :S])

            # out = g @ W7 + b7
            pso = psL.tile([P, CH], F32, tag="mm")
            for k in range(KS):
                nc.tensor.matmul(pso[:S, :NCLS], gT[:, k, :], W7v[:, k, :],
                                 start=(k == 0), stop=False)
            nc.tensor.matmul(pso[:S, :NCLS], ones_row, b7row,
                             start=False, stop=True)
            oc = colpool.tile([S, 16], F32, tag="oc")
            nc.vector.tensor_copy(out=oc[:, :NCLS], in_=pso[:S, :NCLS])
            nc.sync.dma_start(out_d.ap()[:], oc[:, :NCLS])

    nc.compile()
    return nc


def prepare_inputs(x, W1, b1, W2, b2, W3, b3, W4, b4, W5, b5, W6, b6, W7, b7,
                   lengths):
    """Host-side sharding/packing. Returns (in_maps, bins, m_pad)."""
    x = np.ascontiguousarray(np.asarray(x, dtype=np.float32))
    lengths = np.asarray(lengths)
    total = x.shape[0]
    seg_ids = _segment_ids(lengths, total)
    counts = np.bincount(seg_ids, minlength=NSEG).astype(np.int64)
    starts = np.zeros(NSEG + 1, dtype=np.int64)
    starts[1:] = np.cumsum(counts)

    bins = _balance_segments(counts)
    core_frames = [int(sum(counts[s] for s in b)) for b in bins]
    m_pad = ((max(core_frames) + CH - 1) // CH) * CH
    frt = m_pad // P

    W1p = np.zeros((P, HID), dtype=np.float32)
    W1p[:FEAT] = np.asarray(W1, dtype=np.float32)
    W1p[FEAT] = np.asarray(b1, dtype=np.float32)

    def dr_pack(W, dt):
        """[1024, 1024] -> [128, 8, 1024] with Wq[p, k, m] = W[k*128+p, m]."""
        Wf = np.asarray(W, np.float32).reshape(KS, P, HID)
        return np.ascontiguousarray(Wf.transpose(1, 0, 2)).astype(dt)

    misc = np.zeros((P, 32), dtype=np.float32)
    misc[:, MC_B2:MC_B2 + KS] = np.asarray(b2, np.float32).reshape(KS, P).T
    misc[:, MC_B3:MC_B3 + KS] = np.asarray(b3, np.float32).reshape(KS, P).T
    misc[:, MC_B5] = np.float32(np.asarray(b5, np.float32).reshape(-1)[0])

    cbf = np.zeros((P, 128), dtype=np.float32)
    cbf[:, CB_ONES8:CB_ONES8 + 8] = 1.0
    cbf[:SEGS_PER_CORE, CB_ID:CB_ID + SEGS_PER_CORE] = np.eye(
        SEGS_PER_CORE, dtype=np.float32)
    cbf[:, CB_W7:CB_W7 + KS * NCLS] = np.asarray(W7, np.float32).reshape(
        KS, P, NCLS).transpose(1, 0, 2).reshape(P, KS * NCLS)

    rwb = np.zeros((1, 64), dtype=np.float32)
    rwb[0, RW_B7:RW_B7 + NCLS] = np.asarray(b7, np.float32).reshape(-1)
    rwb[0, RW_ONES:RW_ONES + SEGS_PER_CORE] = 1.0

    c8 = np.zeros((1, 2, 1536), dtype=np.float32)
    c8[0, :, 0:P] = 1.0
    b4f = np.asarray(b4, np.float32).reshape(-1)
    b4hi = b4f.astype(E4NP).astype(np.float32)
    b4lo = (b4f - b4hi).astype(E4NP).astype(np.float32)
    c8[0, 0, 512:1536] = b4hi
    c8[0, 1, 512:1536] = b4lo

    shared = dict(
        W1p=W1p.astype(BFNP),
        W2q=dr_pack(W2, E4NP),
        W3q=dr_pack(W3, E4NP),
        W4q=dr_pack(W4, E4NP),
        W5rep=np.broadcast_to(np.asarray(W5, np.float32).reshape(1, HID),
                              (P, HID)).astype(BFNP),
        W6b=dr_pack(W6, BFNP),
        b6r=np.asarray(b6, np.float32).reshape(1, HID).astype(BFNP),
        miscc=misc,
        cbf=cbf.astype(BFNP),
        c8=c8.astype(E4NP),
        rwb=rwb.astype(BFNP),
    )

    in_maps = []
    for core in range(NCORES):
        segs = bins[core]
        xs = [x[starts[s]:starts[s + 1]] for s in segs]
        xcat = np.concatenate(xs, axis=0) if xs else np.zeros((0, FEAT), np.float32)
        n = xcat.shape[0]
        xT = np.zeros((P, m_pad), dtype=np.float32)
        xT[:FEAT, :n] = xcat.T
        xT[FEAT, :n] = 1.0  # constant feature -> b1
        A = np.zeros((m_pad, SEGS_PER_CORE), dtype=np.float32)
        off = 0
        for j, s in enumerate(segs):
            ln = int(counts[s])
            A[off:off + ln, j] = 1.0
            off += ln
        im = dict(shared)
        im["xT"] = xT.astype(BFNP)
        # partition-major layout [P, frt, S]: Ah[p, t, s] = A[t*128 + p, s]
        im["Amat"] = np.ascontiguousarray(
            A.reshape(frt, P, SEGS_PER_CORE).transpose(1, 0, 2)).astype(BFNP)
        in_maps.append(im)
    return in_maps, bins, m_pad


_PROGRAM_CACHE: dict[int, object] = {}


def kernel(**inputs) -> np.ndarray:
    in_maps, bins, m_pad = prepare_inputs(**inputs)
    nc = _PROGRAM_CACHE.get(m_pad)
    if nc is None:
        nc = _build_program(m_pad)
        _PROGRAM_CACHE[m_pad] = nc
    res = run_bass_kernel_spmd(nc, in_maps, core_ids=list(range(NCORES)))
    out = np.zeros((NSEG, NCLS), dtype=np.float32)
    for core in range(NCORES):
        out[bins[core]] = res.results[core]["out"]
    return out


# revision 41
# speedup vs baseline: 1.1453x; 1.0197x over previous
"""Trainium2 Bass kernel for nn_Dnn_with_Attention (ragged attention-pooled DNN).

Contract: kernel(**inputs) takes FULL unsharded numpy inputs (keys as in
reference.setup_inputs()) and returns the FULL [256, 10] float32 output.

Strategy (data-parallel over utterances, 8 NeuronCores):
  - Host: greedily balance the 256 segments over 8 cores (32 whole segments
    each), gather each core's frames, transpose x to feature-major
    bf16 [128(feat-padded), M_PAD] and build a per-frame one-hot segment
    membership matrix A (bf16).  A row of ones is appended as feature 78 so
    b1 folds into W1.
  - Device (per core): L1 in bf16 (feature-major, [1024, frames]); L2/L3/L4
    run in fp8 e4m3 with MatmulPerfMode.DoubleRow (two 128-K slices per
    instruction at 0.5 cycles/row, ~4x the f32r rate).  Weights W2/W3/W4 are
    host-quantized to e4m3; inter-layer activations are written as e4m3
    directly by the relu ops.  L4 produces frame-major h4 in bf16; b4 is
    added via a DoubleRow matmul against a host-packed (hi, lo) e4m3 pair so
    the quantization error cancels.  Scores: GpSimd computes h4 * W5 (SBUF
    only; GPSIMD cannot touch PSUM), DVE reduces the innermost axis, Scalar
    takes the exp; e = max(exp(score + b5), 1) folds the relu.  Segment
    softmax pooling is small PE matmuls E.T @ h4 (E = A * e, bf16)
    accumulated into persistent PSUM across all chunks; the denominator
    comes from E.T @ ones into the same PSUM bank at a different partition
    quadrant.  The final per-utterance MLP runs once at the end in bf16.
  - The whole program is emitted statically as a 6-stage software pipeline
    over chunks (L1 / L2 / L3 / L4+score-product / score-tail / pooling,
    each one iteration apart), so every matmul -> relu -> next-layer
    dependency crosses a full ~19us iteration and the PE never waits on the
    other engines; this also keeps the PE out of its low p-states.  Relu
    drains alternate between Scalar and DVE per psum tile so each layer
    phase drains as fast as the PE fills it.
"""

import sys

sys.path.insert(0, "/opt/trn_rl_repo")

import numpy as np
import ml_dtypes

import concourse.bass as bass
import concourse.mybir as mybir
import concourse.tile as tile
from concourse import bacc
from concourse.bass_utils import run_bass_kernel_spmd

P = 128
FEAT = 78
HID = 1024
NCLS = 10
NSEG = 256
NCORES = 8
SEGS_PER_CORE = NSEG // NCORES
CH = 512           # frames per chunk (free dim of the layer matmuls)
FRT_PER_CH = CH // P
KS = HID // P      # 8 k-subtiles
F32 = mybir.dt.float32
F32R = mybir.dt.float32r
BF16 = mybir.dt.bfloat16
F8 = mybir.dt.float8e4
DR = mybir.MatmulPerfMode.DoubleRow
E4NP = ml_dtypes.float8_e4m3
BFNP = ml_dtypes.bfloat16

# misc constant tile column layout ([128, 32] f32, host-packed)
MC_B2 = 0          # cols 0..7   : b2 striped [128, 8]
MC_B3 = 8          # cols 8..15  : b3 striped
MC_B5 = 17         # col 17      : b5 replicated down partitions
# bf16 const tile ([128, 96])
CB_ONES8 = 0       # cols 0..7  : ones (denom matmul rhs)
CB_ID = 8          # cols 8..39, rows 0..31: 32x32 identity
CB_W7 = 40         # cols 40..119?? keep within 96: W7 as [128, 8, 10] -> 80 cols
# fp8 const row ([1, 2, 1536]): ones pair + b4 (hi, lo) pair
# row layout [1, 2, 1536]: [:, :, 0:128] ones, [:, :, 512:1536] b4 hi/lo
# simpler: two fields side by side, see prepare_inputs
# bf16 row consts ([1, 64])
RW_B7 = 0          # cols 0..9 : b7
RW_ONES = 16       # cols 16..48 : ones row (bias matmuls, final MLP)


def _segment_ids(lengths: np.ndarray, total: int) -> np.ndarray:
    """Replicate jnp.repeat(arange(n), lengths, total_repeat_length=total)."""
    lengths = np.asarray(lengths, dtype=np.int64)
    seg = np.repeat(np.arange(lengths.shape[0], dtype=np.int32), np.maximum(lengths, 0))
    if seg.shape[0] >= total:
        return seg[:total]
    pad_val = seg[-1] if seg.shape[0] > 0 else np.int32(0)
    return np.concatenate([seg, np.full(total - seg.shape[0], pad_val, np.int32)])


def _balance_segments(lengths: np.ndarray) -> list[list[int]]:
    """Assign 256 segments to 8 cores, 32 each, minimizing max frame count."""
    order = np.argsort(-lengths, kind="stable")
    loads = [0] * NCORES
    bins: list[list[int]] = [[] for _ in range(NCORES)]
    for s in order:
        cands = [c for c in range(NCORES) if len(bins[c]) < SEGS_PER_CORE]
        c = min(cands, key=lambda c: (loads[c], c))
        bins[c].append(int(s))
        loads[c] += int(lengths[s])
    for b in bins:
        b.sort()
    return bins


def _build_program(m_pad: int):
    """Emit the Bass/Tile program for one core with m_pad frames (static)."""
    nch = m_pad // CH
    frt = m_pad // P
    S = SEGS_PER_CORE

    nc = bacc.Bacc("TRN2", target_bir_lowering=False, debug=False,
                   num_devices=NCORES)

    xT_d = nc.dram_tensor("xT", [P, m_pad], BF16, kind="ExternalInput")
    A_d = nc.dram_tensor("Amat", [P, frt, S], BF16, kind="ExternalInput")
    W1_d = nc.dram_tensor("W1p", [P, HID], BF16, kind="ExternalInput")
    W2_d = nc.dram_tensor("W2q", [P, KS, HID], F8, kind="ExternalInput")
    W3_d = nc.dram_tensor("W3q", [P, KS, HID], F8, kind="ExternalInput")
    W4_d = nc.dram_tensor("W4q", [P, KS, HID], F8, kind="ExternalInput")
    W5_d = nc.dram_tensor("W5rep", [P, HID], BF16, kind="ExternalInput")
    W6_d = nc.dram_tensor("W6b", [P, KS, HID], BF16, kind="ExternalInput")
    b6_d = nc.dram_tensor("b6r", [1, HID], BF16, kind="ExternalInput")
    misc_d = nc.dram_tensor("miscc", [P, 32], F32, kind="ExternalInput")
    cbf_d = nc.dram_tensor("cbf", [P, 128], BF16, kind="ExternalInput")
    c8_d = nc.dram_tensor("c8", [1, 2, 1536], F8, kind="ExternalInput")
    rw_d = nc.dram_tensor("rwb", [1, 64], BF16, kind="ExternalInput")
    out_d = nc.dram_tensor("out", [S, NCLS], F32, kind="ExternalOutput")

    RELU = mybir.ActivationFunctionType.Relu
    EXP = mybir.ActivationFunctionType.Exp
    MULT = mybir.AluOpType.mult
    ADD = mybir.AluOpType.add
    MAX = mybir.AluOpType.max

    with tile.TileContext(nc) as tc:
        with (
            tc.tile_pool(name="wpool", bufs=1) as wpool,
            tc.tile_pool(name="xpool", bufs=6) as xpool,
            tc.tile_pool(name="apool", bufs=10) as apool,
            tc.tile_pool(name="h1pool", bufs=2) as h1pool,
            tc.tile_pool(name="h2pool", bufs=2) as h2pool,
            tc.tile_pool(name="h3pool", bufs=2) as h3pool,
            tc.tile_pool(name="h4pool", bufs=4) as h4pool,
            tc.tile_pool(name="scrpool", bufs=4) as scrpool,
            tc.tile_pool(name="colpool", bufs=4) as colpool,
            tc.tile_pool(name="epool", bufs=4) as epool,
            tc.tile_pool(name="fpool", bufs=1) as fpool,
            tc.tile_pool(name="psL", bufs=7, space="PSUM") as psL,
            tc.tile_pool(name="psAcc", bufs=1, space="PSUM") as psAcc,
        ):
            # ---- resident constants/weights ----
            W1s = wpool.tile([P, HID], BF16, tag="W1")
            nc.sync.dma_start(W1s[:], W1_d.ap())

            def load_w(d, tagp, dt):
                t = wpool.tile([P, KS, HID], dt, tag=tagp)
                for k in range(KS):
                    nc.sync.dma_start(t[:, k, :], d.ap()[:, k, :])
                return t

            misc = wpool.tile([P, 32], F32, tag="misc")
            nc.sync.dma_start(misc[:], misc_d.ap())
            cbf = wpool.tile([P, 128], BF16, tag="cbf")
            nc.sync.dma_start(cbf[:], cbf_d.ap())
            c8 = wpool.tile([1, 2, 1536], F8, tag="c8")
            nc.sync.dma_start(c8[:], c8_d.ap())
            rwb = wpool.tile([1, 64], BF16, tag="rwb")
            nc.sync.dma_start(rwb[:], rw_d.ap())

            # prefetch the first six chunks' x/A BEFORE the big weight
            # loads: L1 of chunk 0 only needs W1 + x(0), so the PE can
            # start ~2us in instead of waiting ~21us for all weights.
            pre_ = {}
            for c0 in range(min(6, nch)):
                xt0 = xpool.tile([P, CH], BF16, tag="x")
                nc.sync.dma_start(xt0[:], xT_d.ap()[:, c0 * CH:(c0 + 1) * CH])
                ag0 = apool.tile([P, FRT_PER_CH, S], BF16, tag="A")
                nc.sync.dma_start(
                    ag0[:], A_d.ap()[:, c0 * FRT_PER_CH:
                                     (c0 + 1) * FRT_PER_CH, :])
                pre_[c0] = {"ag": ag0, "xt": xt0}

            W2s = load_w(W2_d, "W2q", F8)
            W3s = load_w(W3_d, "W3q", F8)
            W4s = load_w(W4_d, "W4q", F8)
            W5s4 = wpool.tile([P, FRT_PER_CH, HID], BF16, tag="W5")
            for f in range(FRT_PER_CH):
                nc.sync.dma_start(W5s4[:, f, :], W5_d.ap())

            b5col = misc[:, MC_B5:MC_B5 + 1]
            ones8 = cbf[:, CB_ONES8:CB_ONES8 + 8]
            ident = cbf[:S, CB_ID:CB_ID + S]
            W7v = cbf[:, CB_W7:CB_W7 + KS * NCLS].rearrange(
                "p (o c) -> p o c", c=NCLS)
            b7row = rwb[:, RW_B7:RW_B7 + NCLS]
            ones_row = rwb[:, RW_ONES:RW_ONES + S]
            ones_pair8 = c8[:, :, 0:P]          # [1, 2, 128] of ones (fp8)
            b4pair = c8[:, :, 512:1536]         # [1, 2, 1024] b4 (hi, lo)

            # persistent PSUM accumulators, all in ONE bank at different
            # partition quadrants (frees a bank for deeper psum rotation):
            #   pooled0 [0:32, 0:512], pooled1 [32:64, 0:512],
            #   denom [64:96, 0:8]
            acc0 = psAcc.tile([P, 512], F32, tag="acc0")
            pooled0 = acc0[0:S, :]
            pooled1 = acc0[S:2 * S, :]
            denom = acc0[2 * S:3 * S, 0:8]

            # ---- main pass: 5-stage software pipeline over chunks ----
            # Stage k of chunk c runs in iteration c+k, so every cross-engine
            # dependency (matmul -> relu -> next layer's matmul) has a full
            # iteration (~15us) of slack and the PE never waits on the relus.
            st_ = {}   # per-chunk tile state

            def relu_ps(out, in_, bias, eng):
                """relu(in_ + bias) -> out (fp8/bf16 cast) on Scalar or DVE."""
                if eng == 0:
                    nc.scalar.activation(out, in_, RELU,
                                         bias=0.0 if bias is None else bias)
                elif bias is None:
                    nc.vector.tensor_scalar_max(out, in_, 0.0)
                else:
                    nc.vector.tensor_scalar(out=out, in0=in_, scalar1=bias,
                                            scalar2=0.0, op0=ADD, op1=MAX)

            def s0(c):  # prefetch x/A one iteration ahead of the L1 use
                if c in pre_:
                    st_[c] = pre_[c]
                    return
                xt = xpool.tile([P, CH], BF16, tag="x")
                nc.sync.dma_start(xt[:], xT_d.ap()[:, c * CH:(c + 1) * CH])
                ag = apool.tile([P, FRT_PER_CH, S], BF16, tag="A")
                nc.sync.dma_start(
                    ag[:], A_d.ap()[:, c * FRT_PER_CH:(c + 1) * FRT_PER_CH, :])
                st_[c] = {"ag": ag, "xt": xt}

            def s1(c):  # L1 (bf16) -> h1 fp8 (per-m relu, alternating eng)
                xt = st_[c]["xt"]
                h1 = h1pool.tile([P, KS, CH], F8, tag="h1")
                for m in range(KS):
                    ps = psL.tile([P, CH], F32, tag="mm")
                    nc.tensor.matmul(ps[:], W1s[:, m * P:(m + 1) * P], xt[:],
                                     start=True, stop=True)
                    relu_ps(h1[:, m, :], ps[:], None,
                            0 if m in (0, 1, 2, 4, 6, 7) else 1)
                st_[c]["h1"] = h1

            def _mid_layer(c, Ws, hin_key, hout_key, pool, boff, flip):
                """L2/L3: fp8 DoubleRow + per-m relu(+bias), alternating
                engines per psum-tile half so the drain keeps up with PE."""
                hin = st_[c][hin_key]
                hout = pool.tile([P, KS, CH], F8, tag=hout_key)
                for m in range(KS):
                    ps = psL.tile([P, CH], F32, tag="mm")
                    for t in range(KS // 2):
                        nc.tensor.matmul(
                            ps[:],
                            Ws[:, 2 * t:2 * t + 2, m * P:(m + 1) * P],
                            hin[:, 2 * t:2 * t + 2, :],
                            start=(t == 0), stop=(t == KS // 2 - 1),
                            perf_mode=DR)
                    relu_ps(hout[:, m, :], ps[:],
                            misc[:, boff + m:boff + m + 1],
                            1 if (m + flip) % 8 in (1, 4, 6) else 0)
                st_[c][hout_key] = hout

            def s2(c):
                _mid_layer(c, W2s, "h1", "h2", h2pool, MC_B2, 0)

            def s3(c):
                _mid_layer(c, W3s, "h2", "h3", h3pool, MC_B3, 1)

            def s4(c):  # L4 fp8 DoubleRow -> h4 bf16; scores -> E (bf16)
                h3 = st_[c]["h3"]
                ag = st_[c]["ag"]
                h4 = h4pool.tile([P, FRT_PER_CH, HID], BF16, tag="h4")
                for f in range(FRT_PER_CH):
                    for n in range(2):
                        ps4 = psL.tile([P, CH], F32, tag="mm")
                        nc.tensor.matmul(ps4[:], ones_pair8,
                                         b4pair[:, :, n * 512:(n + 1) * 512],
                                         start=True, stop=False, perf_mode=DR)
                        for t in range(KS // 2):
                            nc.tensor.matmul(
                                ps4[:],
                                h3[:, 2 * t:2 * t + 2, f * P:(f + 1) * P],
                                W4s[:, 2 * t:2 * t + 2, n * 512:(n + 1) * 512],
                                start=False, stop=(t == KS // 2 - 1),
                                perf_mode=DR)
                        relu_ps(h4[:, f, n * 512:(n + 1) * 512], ps4[:], None,
                                0 if (2 * f + n) in (0, 1, 2, 4, 6, 7) else 1)
                # scores, fully batched: one GpSimd product over all four
                # f-tiles, one DVE reduce (innermost axis), one exp, one
                # clamp; per-f E columns on GpSimd.  E is consumed by the
                # pooling matmuls one iteration later.
                scr = scrpool.tile([P, FRT_PER_CH, HID], BF16, tag="scr")
                for f in range(FRT_PER_CH):
                    # drain tail (last two chunks): the product runs on DVE
                    # in the fast bf16 2x mode -- once the main loop ends
                    # nothing hides the serial GpSimd product chain
                    nc.vector.tensor_mul(scr[:, f, :], h4[:, f, :],
                                         W5s4[:, f, :])
                st_[c]["h4"] = h4
                st_[c]["scr"] = scr

            def s4b(c):  # score reduces at the very END of the DVE stream
                # (their products are an iteration old, so they never
                # head-of-line-block the DVE act drains).
                scr = st_[c]["scr"]
                ct = colpool.tile([P, FRT_PER_CH], F32, tag="ctb")
                for f in range(FRT_PER_CH):
                    nc.vector.tensor_reduce(out=ct[:, f:f + 1],
                                            in_=scr[:, f, :],
                                            axis=mybir.AxisListType.X, op=ADD)
                st_[c]["ct"] = ct

            def s4c(c):  # exp/clamp/E at the START of the next iteration:
                # exp leads the Scalar queue (its reduces are an iteration
                # old) so Pool's clamp/E never wait deep into the iteration
                # and Pool's next product batch starts on time.
                ag = st_[c]["ag"]
                ct = st_[c]["ct"]
                etg = epool.tile([P, FRT_PER_CH, S], BF16, tag="E")
                ec = colpool.tile([P, 2 * FRT_PER_CH], F32, tag="ec")
                nc.scalar.activation(ec[:, :FRT_PER_CH], ct[:], EXP,
                                     bias=b5col)
                nc.gpsimd.tensor_scalar_max(ec[:, FRT_PER_CH:],
                                            ec[:, :FRT_PER_CH], 1.0)
                for f in range(FRT_PER_CH):
                    nc.gpsimd.tensor_scalar_mul(
                        etg[:, f, :], ag[:, f, :],
                        ec[:, FRT_PER_CH + f:FRT_PER_CH + f + 1])
                st_[c]["et"] = etg

            def s5(c):  # pooling matmuls (persistent PSUM accumulation)
                h4 = st_[c]["h4"]
                etg = st_[c]["et"]
                first = c == 0
                last = c == nch - 1
                for f in range(FRT_PER_CH):
                    et = etg[:, f, :]
                    st = bool(first and f == 0)
                    sp = bool(last and f == FRT_PER_CH - 1)
                    # pooled0/denom share a PSUM bank at different partition
                    # quadrants; the sim's group check is partition-blind so
                    # it must be skipped (values verified exact in CoreSim).
                    nc.tensor.matmul(pooled0, et, h4[:, f, :512],
                                     start=st, stop=sp, skip_group_check=True)
                    nc.tensor.matmul(pooled1, et, h4[:, f, 512:],
                                     start=st, stop=sp, skip_group_check=True)
                    nc.tensor.matmul(denom, et, ones8,
                                     start=st, stop=sp, skip_group_check=True)
                del st_[c]

            sched = ((s4c, 5), (s0, -1), (s1, 0), (s2, 1), (s3, 2),
                     (s4, 3), (s4b, 4), (s5, 5))
            for i in range(-1, nch + 5):
                for stage, off in sched:
                    c = i - off
                    if 0 <= c < nch:
                        stage(c)

            # ---- final per-utterance MLP ----
            W6s = load_w(W6_d, "W6b", BF16)
            b6s = wpool.tile([1, HID], BF16, tag="b6")
            nc.sync.dma_start(b6s[:], b6_d.ap())

            # 1/denom: copy the [64:96] psum quadrant to SBUF, DMA-shift it
            # to partitions 0:32 and 32:64 (engines can't move across lanes)
            dtmp = fpool.tile([3 * S, 1], F32, tag="dtmp")
            nc.vector.tensor_copy(out=dtmp[2 * S:3 * S, 0:1],
                                  in_=denom[:, 0:1])
            fc = colpool.tile([2 * S, 4], F32, tag="col")
            nc.sync.dma_start(fc[0:S, 0:1], dtmp[2 * S:3 * S, 0:1])
            nc.sync.dma_start(fc[S:2 * S, 0:1], dtmp[2 * S:3 * S, 0:1])
            nc.vector.reciprocal(fc[:, 1:2], fc[:, 0:1])

            # pooled (normalized) in f32 for the PE transpose; pooled1 is
            # scaled in place at partitions 32:64, then DMA-shifted down
            pooled_sb = fpool.tile([S, HID], F32, tag="pooled")
            pstg = fpool.tile([2 * S, 512], F32, tag="pstg")
            nc.vector.tensor_scalar_mul(pooled_sb[:, :512], pooled0,
                                        fc[0:S, 1:2])
            nc.vector.tensor_scalar_mul(pstg[S:2 * S, :], pooled1,
                                        fc[S:2 * S, 1:2])
            nc.sync.dma_start(pooled_sb[:, 512:], pstg[S:2 * S, :])

            # transpose pooled -> pooledT [hid, seg] (bf16 via cast copies)
            identf = fpool.tile([S, S], F32, tag="identf")
            nc.vector.tensor_copy(out=identf[:], in_=ident)
            tposed = fpool.tile([P, KS, 2 * S], BF16, tag="tposed")
            pooledT = tposed[:, :, :S]
            gT = tposed[:, :, S:]
            for k in range(KS):
                pst = psL.tile([P, CH], F32, tag="mm")
                nc.tensor.transpose(pst[:, :S], pooled_sb[:, k * P:(k + 1) * P],
                                    identf[:])
                nc.vector.tensor_copy(out=pooledT[:, k, :], in_=pst[:, :S])

            # g = relu(pooled @ W6 + b6)   (seg-major [S, HID], bf16)
            g_sb = fpool.tile([S, HID], BF16, tag="g")
            for n in range(2):
                psg = psL.tile([P, CH], F32, tag="mm")
                for k in range(KS):
                    nc.tensor.matmul(psg[:S, :], pooledT[:, k, :],
                                     W6s[:, k, n * 512:(n + 1) * 512],
                                     start=(k == 0), stop=False)
                nc.tensor.matmul(psg[:S, :], ones_row,
                                 b6s[:, n * 512:(n + 1) * 512],
                                 start=False, stop=True)
                nc.scalar.activation(g_sb[:, n * 512:(n + 1) * 512],
                                     psg[:S, :], RELU)

            # gT [hid, seg] (transpose back via f32 staging)
            gf = fpool.tile([S, HID], F32, tag="gf")
            nc.vector.tensor_copy(out=gf[:], in_=g_sb[:])
            for k in range(KS):
                pst = psL.tile([P, CH], F32, tag="mm")
                nc.tensor.transpose(pst[:, :S], gf[:, k * P:(k + 1) * P],
                                    identf[:])
                nc.vector.tensor_copy(out=gT[:, k, :], in_=pst[:, :S])

            # out = g @ W7 + b7
            pso = psL.tile([P, CH], F32, tag="mm")
            for k in range(KS):
                nc.tensor.matmul(pso[:S, :NCLS], gT[:, k, :], W7v[:, k, :],
                                 start=(k == 0), stop=False)
            nc.tensor.matmul(pso[:S, :NCLS], ones_row, b7row,
                             start=False, stop=True)
            oc = colpool.tile([S, 16], F32, tag="oc")
            nc.vector.tensor_copy(out=oc[:, :NCLS], in_=pso[:S, :NCLS])
            nc.sync.dma_start(out_d.ap()[:], oc[:, :NCLS])

    nc.compile()
    return nc


def prepare_inputs(x, W1, b1, W2, b2, W3, b3, W4, b4, W5, b5, W6, b6, W7, b7,
                   lengths):
    """Host-side sharding/packing. Returns (in_maps, bins, m_pad)."""
    x = np.ascontiguousarray(np.asarray(x, dtype=np.float32))
    lengths = np.asarray(lengths)
    total = x.shape[0]
    seg_ids = _segment_ids(lengths, total)
    counts = np.bincount(seg_ids, minlength=NSEG).astype(np.int64)
    starts = np.zeros(NSEG + 1, dtype=np.int64)
    starts[1:] = np.cumsum(counts)

    bins = _balance_segments(counts)
    core_frames = [int(sum(counts[s] for s in b)) for b in bins]
    m_pad = ((max(core_frames) + CH - 1) // CH) * CH
    frt = m_pad // P

    W1p = np.zeros((P, HID), dtype=np.float32)
    W1p[:FEAT] = np.asarray(W1, dtype=np.float32)
    W1p[FEAT] = np.asarray(b1, dtype=np.float32)

    def dr_pack(W, dt):
        """[1024, 1024] -> [128, 8, 1024] with Wq[p, k, m] = W[k*128+p, m]."""
        Wf = np.asarray(W, np.float32).reshape(KS, P, HID)
        return np.ascontiguousarray(Wf.transpose(1, 0, 2)).astype(dt)

    misc = np.zeros((P, 32), dtype=np.float32)
    misc[:, MC_B2:MC_B2 + KS] = np.asarray(b2, np.float32).reshape(KS, P).T
    misc[:, MC_B3:MC_B3 + KS] = np.asarray(b3, np.float32).reshape(KS, P).T
    misc[:, MC_B5] = np.float32(np.asarray(b5, np.float32).reshape(-1)[0])

    cbf = np.zeros((P, 128), dtype=np.float32)
    cbf[:, CB_ONES8:CB_ONES8 + 8] = 1.0
    cbf[:SEGS_PER_CORE, CB_ID:CB_ID + SEGS_PER_CORE] = np.eye(
        SEGS_PER_CORE, dtype=np.float32)
    cbf[:, CB_W7:CB_W7 + KS * NCLS] = np.asarray(W7, np.float32).reshape(
        KS, P, NCLS).transpose(1, 0, 2).reshape(P, KS * NCLS)

    rwb = np.zeros((1, 64), dtype=np.float32)
    rwb[0, RW_B7:RW_B7 + NCLS] = np.asarray(b7, np.float32).reshape(-1)
    rwb[0, RW_ONES:RW_ONES + SEGS_PER_CORE] = 1.0

    c8 = np.zeros((1, 2, 1536), dtype=np.float32)
    c8[0, :, 0:P] = 1.0
    b4f = np.asarray(b4, np.float32).reshape(-1)
    b4hi = b4f.astype(E4NP).astype(np.float32)
    b4lo = (b4f - b4hi).astype(E4NP).astype(np.float32)
    c8[0, 0, 512:1536] = b4hi
    c8[0, 1, 512:1536] = b4lo

    shared = dict(
        W1p=W1p.astype(BFNP),
        W2q=dr_pack(W2, E4NP),
        W3q=dr_pack(W3, E4NP),
        W4q=dr_pack(W4, E4NP),
        W5rep=np.broadcast_to(np.asarray(W5, np.float32).reshape(1, HID),
                              (P, HID)).astype(BFNP),
        W6b=dr_pack(W6, BFNP),
        b6r=np.asarray(b6, np.float32).reshape(1, HID).astype(BFNP),
        miscc=misc,
        cbf=cbf.astype(BFNP),
        c8=c8.astype(E4NP),
        rwb=rwb.astype(BFNP),
    )

    in_maps = []
    for core in range(NCORES):
        segs = bins[core]
        xs = [x[starts[s]:starts[s + 1]] for s in segs]
        xcat = np.concatenate(xs, axis=0) if xs else np.zeros((0, FEAT), np.float32)
        n = xcat.shape[0]
        xT = np.zeros((P, m_pad), dtype=np.float32)
        xT[:FEAT, :n] = xcat.T
        xT[FEAT, :n] = 1.0  # constant feature -> b1
        A = np.zeros((m_pad, SEGS_PER_CORE), dtype=np.float32)
        off = 0
        for j, s in enumerate(segs):
            ln = int(counts[s])
            A[off:off + ln, j] = 1.0
            off += ln
        im = dict(shared)
        im["xT"] = xT.astype(BFNP)
        # partition-major layout [P, frt, S]: Ah[p, t, s] = A[t*128 + p, s]
        im["Amat"] = np.ascontiguousarray(
            A.reshape(frt, P, SEGS_PER_CORE).transpose(1, 0, 2)).astype(BFNP)
        in_maps.append(im)
    return in_maps, bins, m_pad


_PROGRAM_CACHE: dict[int, object] = {}


def kernel(**inputs) -> np.ndarray:
    in_maps, bins, m_pad = prepare_inputs(**inputs)
    nc = _PROGRAM_CACHE.get(m_pad)
    if nc is None:
        nc = _build_program(m_pad)
        _PROGRAM_CACHE[m_pad] = nc
    res = run_bass_kernel_spmd(nc, in_maps, core_ids=list(range(NCORES)))
    out = np.zeros((NSEG, NCLS), dtype=np.float32)
    for core in range(NCORES):
        out[bins[core]] = res.results[core]["out"]
    return out
